# revision 16
# baseline (speedup 1.0000x reference)
"""DINOv3 ViT attention (RoPE + det-temp scaling + additive gate) on 8 TRN2 cores.

Sharding: pure data-parallel over batch (B=8 -> 1 batch element per core).
Weights / gate / rope tables replicated. No collectives.

v3 design (engine budget per the TimelineSim cost model):
  - gate folded multiplicatively: EG = exp(gate^T) precomputed on host (bf16);
    after the ACT exp of raw scores, one DVE bf16 multiply applies it
    (replaces the per-head identity-matmul gate copies on PE).
  - scores per (head, skt): qT bf16 (moving operand -> 1 cyc/col at any
    width), kT bf16 (stationary), full-sq psum [128, 1152] in 3 chunks.
  - exp on ACT in 1-2 wide ops per (head, skt) (amortizes the ~185ns per-op
    access-latency penalty).
  - ctx flipped: out[sq_tile, 65] = e'^T @ v  (lhsT = e' bf16), N=65 per
    matmul -> half the PE columns of the [hd, sq] orientation; the v ones
    column lands the softmax denominator as a per-partition column, so
    normalization is one strided DVE reciprocal + 9 GpSimd scalar muls.
  - ctx [sq, hd] bf16 -> ctxT [hd, sq] via DMA XBAR transposes (idle engine).
  - phase merge: attention for head pair p starts as soon as kT/qT tile p is
    finalized; remaining projection/RoPE work is emitted as PE filler between
    attention steps.  While projection psum pools are open the score psum is
    single-buffered (8-bank budget) and the exp is split in two so the
    next-step score matmuls unblock early; once projections finish, a second
    score psum opens and exps run whole.
"""
import numpy as np
from contextlib import ExitStack

import ml_dtypes
import concourse.bacc as bacc
import concourse.mybir as mybir
import concourse.tile as tile
from concourse.bass_utils import run_bass_kernel_spmd

F32 = mybir.dt.float32
F32R = mybir.dt.float32r
BF16 = mybir.dt.bfloat16
AF = mybir.ActivationFunctionType

# ---------------- problem config (hardcoded per harness contract) ------------


class CFG:
    B = 8
    S = 1129
    SP = 1152            # padded S (9 * 128)
    D = 768
    H = 12
    HD = 64
    ROPE_START = 5
    ROPE_END = 1029
    DET_START = 1029
    DET_END = 1129
    P_SCALE = 2.0
    N_CORES = 8
    SQB = 384            # projection psum chunk
    GATE_NEG = -30.0     # gate value for pad keys: exp(-30) ~ 9e-14
    CTX_STRIDE = 74      # ctx psum window stride (65-wide windows, no
                         # 512-col psum bank crossings for 9 windows)
    FILLER_EVERY = 2     # pop one projection filler every N attention steps

    @property
    def KT(self):
        return self.D // 128          # dout/din 128-tiles (6)

    @property
    def NT(self):
        return self.SP // 128         # s 128-tiles (9)

    @property
    def ROPE_LEN(self):
        return self.ROPE_END - self.ROPE_START


def round_f32r(x: np.ndarray) -> np.ndarray:
    """Round fp32 to the fp32r format (11 mantissa bits, RNE)."""
    b = np.ascontiguousarray(x, dtype=np.float32).view(np.uint32)
    low = b & np.uint32(0xFFF)
    b = b & np.uint32(0xFFFFF000)
    rnd = (low > 0x800) | ((low == 0x800) & (((b >> 12) & 1) != 0))
    b = b + (rnd.astype(np.uint32) << 12)
    return b.view(np.float32)


def to_bf16(x: np.ndarray) -> np.ndarray:
    return np.ascontiguousarray(x, dtype=np.float32).astype(ml_dtypes.bfloat16)


# ---------------- device program ------------------------------------------


def build_nc(cfg: CFG):
    nc = bacc.Bacc(trn_type="TRN2", target_bir_lowering=False, debug=False)
    KT, NT, SQB, SP = cfg.KT, cfg.NT, cfg.SQB, cfg.SP
    H, HD = cfg.H, cfg.HD
    RS, RE, DS, DE = cfg.ROPE_START, cfg.ROPE_END, cfg.DET_START, cfg.DET_END
    RL = cfg.ROPE_LEN
    DET = DE - DS
    S = cfg.S
    CS = cfg.CTX_STRIDE

    # ---- dram parameters (per core) ----
    d_hsT = nc.dram_tensor("hsT", [cfg.D, SP], BF16, kind="ExternalInput").ap()
    d_qwT = nc.dram_tensor("qwT", [cfg.D, cfg.D], F32R, kind="ExternalInput").ap()
    d_kwT = nc.dram_tensor("kwT", [cfg.D, cfg.D], F32R, kind="ExternalInput").ap()
    d_vwT = nc.dram_tensor("vwT", [(KT + 1) * 128, cfg.D], F32R, kind="ExternalInput").ap()
    d_owT = nc.dram_tensor("owT", [cfg.D, cfg.D], F32R, kind="ExternalInput").ap()
    d_EG = nc.dram_tensor("EG", [SP, SP], BF16, kind="ExternalInput").ap()
    d_qb = nc.dram_tensor("qb", [128, KT], F32, kind="ExternalInput").ap()
    d_ob = nc.dram_tensor("ob", [128, cfg.D], F32, kind="ExternalInput").ap()
    d_cosT2 = nc.dram_tensor("cosT2", [128, RL], F32, kind="ExternalInput").ap()
    d_sinT2 = nc.dram_tensor("sinT2", [128, RL], F32, kind="ExternalInput").ap()
    d_rotT = nc.dram_tensor("rotT", [128, 128], F32R, kind="ExternalInput").ap()
    d_ones = nc.dram_tensor("ones", [1, 128], F32R, kind="ExternalInput").ap()
    d_onescol = nc.dram_tensor("onescol", [128, H], BF16, kind="ExternalInput").ap()
    d_masks = nc.dram_tensor("masks", [1, 256], F32R, kind="ExternalInput").ap()
    d_ph = nc.dram_tensor("ph", [1, DET], F32, kind="ExternalInput").ap()
    d_pw = nc.dram_tensor("pw", [1, DET], F32, kind="ExternalInput").ap()
    d_out = nc.dram_tensor("out", [SP, cfg.D], F32, kind="ExternalOutput").ap()

    with tile.TileContext(nc) as tc, ExitStack() as gctx:
        # ---------------- global sbuf (spans the whole kernel) --------------
        gsb = gctx.enter_context(tc.tile_pool(name="gsb", bufs=1))

        t_ones = gsb.tile([1, 128], F32R, tag="ones")
        nc.sync.dma_start(t_ones[:], d_ones[:, :])

        # persistent activations
        t_qTf = [gsb.tile([128, SP], BF16, tag=f"qTf{m}", name=f"qTf{m}") for m in range(KT)]
        t_kTf = [gsb.tile([128, SP], BF16, tag=f"kTf{m}", name=f"kTf{m}") for m in range(KT)]
        t_v = [gsb.tile([128, H * 65], BF16, tag=f"v{t}", name=f"v{t}") for t in range(NT)]
        t_EG = [gsb.tile([128, SP], BF16, tag=f"eg{t}", name=f"eg{t}") for t in range(NT)]
        NE = 4
        t_e = [gsb.tile([128, SP], BF16, tag=f"e{i}", name=f"e{i}") for i in range(NE)]
        t_ctx = [gsb.tile([128, cfg.D], BF16, tag=f"ctx{t}", name=f"ctx{t}") for t in range(NT)]
        t_ctxT = [gsb.tile([128, SP], BF16, tag=f"ctxT{k}", name=f"ctxT{k}") for k in range(KT)]
        t_rec = [gsb.tile([128, NT], F32, tag=f"rec{i}", name=f"rec{i}") for i in range(2)]
        t_tem = gsb.tile([128, DET], F32, tag="tem")

        # ---- det temperature pattern (standalone psum stack) ----
        with ExitStack() as ptem:
            sbt = ptem.enter_context(tc.tile_pool(name="sbt", bufs=1))
            ps_tem = ptem.enter_context(tc.tile_pool(name="ps_tem", bufs=1, space="PSUM"))
            t_ms = sbt.tile([1, 256], F32R, tag="ms")
            nc.sync.dma_start(t_ms[:], d_masks[:, :])
            t_ph = sbt.tile([1, DET], F32, tag="ph")
            nc.sync.dma_start(t_ph[:], d_ph[:, :])
            t_pw = sbt.tile([1, DET], F32, tag="pw")
            nc.sync.dma_start(t_pw[:], d_pw[:, :])
            t_eh = sbt.tile([1, DET], F32R, tag="eh")
            nc.scalar.activation(t_eh[:], t_ph[:], AF.Exp, bias=0.0, scale=cfg.P_SCALE)
            t_ew = sbt.tile([1, DET], F32R, tag="ew")
            nc.scalar.activation(t_ew[:], t_pw[:], AF.Exp, bias=0.0, scale=cfg.P_SCALE)
            p_tem = ps_tem.tile([128, DET], F32, tag="tem")
            nc.tensor.matmul(p_tem[:], t_ms[0:1, 0:128], t_eh[:], start=True, stop=False)
            nc.tensor.matmul(p_tem[:], t_ms[0:1, 128:256], t_ew[:], start=False, stop=True)
            nc.vector.tensor_copy(t_tem[:], p_tem[:])

        # ==================== merged projections + attention ================
        with ExitStack() as att:
            sb3 = att.enter_context(tc.tile_pool(name="sb3", bufs=1))
            t_ob = sb3.tile([128, cfg.D], F32, tag="ob")
            t_ow = [sb3.tile([128, cfg.D], F32R, tag=f"ow{k}", name=f"ow{k}")
                    for k in range(KT)]
            apsum = att.enter_context(ExitStack())
            ps_scA = apsum.enter_context(tc.tile_pool(name="ps_scA", bufs=1, space="PSUM"))
            ps_ctx = apsum.enter_context(tc.tile_pool(name="ps_ctx", bufs=1, space="PSUM"))

            pB = ExitStack()
            sb1 = pB.enter_context(tc.tile_pool(name="sb1", bufs=1))
            wsb = pB.enter_context(tc.tile_pool(name="wsb", bufs=1))
            ps_q = pB.enter_context(tc.tile_pool(name="ps_q", bufs=2, space="PSUM"))
            ps_rot = pB.enter_context(tc.tile_pool(name="ps_rot", bufs=1, space="PSUM"))

            # ---- loads: hsT/kw interleaved first (k-proj is the prelude) ----
            vst = ExitStack()
            vsb = vst.enter_context(tc.tile_pool(name="vsb", bufs=1))

            t_hsT = []
            kw = []
            for k in range(KT):
                t = sb1.tile([128, SP], BF16, tag=f"hsT{k}", name=f"hsT{k}")
                eng = nc.sync if k % 2 == 0 else nc.scalar
                eng.dma_start(t[:], d_hsT[k * 128:(k + 1) * 128, :])
                t_hsT.append(t)
                w = wsb.tile([128, cfg.D], F32R, tag=f"w{k}", name=f"kw{k}")
                weng = nc.scalar if k % 2 == 0 else nc.sync
                weng.dma_start(w[:], d_kwT[k * 128:(k + 1) * 128, :])
                kw.append(w)
            t_cos = sb1.tile([128, RL], F32, tag="cos")
            nc.scalar.dma_start(t_cos[:], d_cosT2[:, :])
            t_sin = sb1.tile([128, RL], F32, tag="sin")
            nc.scalar.dma_start(t_sin[:], d_sinT2[:, :])
            t_rotT = sb1.tile([128, 128], F32R, tag="rotT")
            nc.sync.dma_start(t_rotT[:], d_rotT[:, :])
            t_qb = sb1.tile([128, KT], F32, tag="qb")
            nc.sync.dma_start(t_qb[:], d_qb[:, :])
            qwl = []
            for k in range(KT):
                w = sb1.tile([128, cfg.D], F32R, tag=f"qw{k}", name=f"qw{k}")
                (nc.sync if k % 2 == 0 else nc.scalar).dma_start(
                    w[:], d_qwT[k * 128:(k + 1) * 128, :])
                qwl.append(w)
            vwl = []
            for k in range(KT):
                w = vsb.tile([128, cfg.D], F32R, tag=f"vw{k}", name=f"vw{k}")
                (nc.sync if k % 2 == 0 else nc.scalar).dma_start(
                    w[:], d_vwT[k * 128:(k + 1) * 128, :])
                vwl.append(w)
            t_vb = sb1.tile([1, cfg.D], F32R, tag="vb")
            nc.sync.dma_start(t_vb[:], d_vwT[cfg.D:cfg.D + 1, :])
            t_onescol = sb1.tile([128, H], BF16, tag="onescol")
            nc.sync.dma_start(t_onescol[:], d_onescol[:, :])
            for t in range(NT):
                nc.scalar.dma_start(t_EG[t][:], d_EG[t * 128:(t + 1) * 128, :])
            nc.sync.dma_start(t_ob[:], d_ob[:, :])
            for k in range(KT):
                nc.sync.dma_start(t_ow[k][:], d_owT[k * 128:(k + 1) * 128, :])

            # ---- projection / finalize emitters ----
            def fin_half(dst, c0):
                """RoPE on dst[:, RS+c0 : RS+c0+512] in place (one psum chunk)."""
                p_rot = ps_rot.tile([128, 512], F32, tag="rot")
                nc.tensor.matmul(p_rot[:], t_rotT[:], dst[:, RS + c0:RS + c0 + 512],
                                 start=True, stop=True)
                tmp1 = sb1.tile([128, 512], F32, tag="tmp1", bufs=2)
                nc.vector.tensor_mul(tmp1[:], p_rot[:], t_sin[:, c0:c0 + 512])
                nc.gpsimd.tensor_mul(dst[:, RS + c0:RS + c0 + 512],
                                     dst[:, RS + c0:RS + c0 + 512],
                                     t_cos[:, c0:c0 + 512])
                nc.vector.tensor_add(dst[:, RS + c0:RS + c0 + 512],
                                     dst[:, RS + c0:RS + c0 + 512], tmp1[:])

            def fin_det(dst):
                nc.gpsimd.tensor_mul(dst[:, DS:DE], dst[:, DS:DE], t_tem[:])

            def proj_chunk(dst_tile, wlist, m, nb0, is_q):
                p = ps_q.tile([128, SQB], F32, tag="qp")
                for k in range(KT):
                    nc.tensor.matmul(p[:], wlist[k][:, m * 128:(m + 1) * 128],
                                     t_hsT[k][:, nb0:nb0 + SQB],
                                     start=(k == 0), stop=(k == KT - 1))
                if is_q:
                    nc.vector.tensor_scalar_add(dst_tile[:, nb0:nb0 + SQB], p[:],
                                                t_qb[:, m:m + 1])
                else:
                    nc.scalar.copy(dst_tile[:, nb0:nb0 + SQB], p[:])

            def v_chunk(mt, n0):
                p = ps_q.tile([128, SQB], F32, tag="qp", name=f"vp{mt}_{n0}")
                nc.tensor.matmul(p[:], t_ones[0:1, :], t_vb[0:1, n0:n0 + SQB],
                                 start=True, stop=False)
                for k in range(KT):
                    nc.tensor.matmul(p[:], t_hsT[k][:, mt * 128:(mt + 1) * 128],
                                     vwl[k][:, n0:n0 + SQB],
                                     start=False, stop=(k == KT - 1))
                nh = SQB // HD  # heads covered by this chunk (6)
                h0 = n0 // HD
                vin = p[:, :].rearrange("p (h j) -> p h j", h=nh)
                v3 = t_v[mt][:, h0 * 65:(h0 + nh) * 65].rearrange(
                    "p (h j) -> p h j", j=65)
                nc.scalar.activation(v3[:, :, 0:HD], vin, AF.Identity,
                                     bias=0.0, scale=1.0)

            def tile_work(m):
                """Filler units (closures) building kT[m], qT[m] + finalize."""
                units = []
                for nb0 in range(0, SP, SQB):
                    units.append(lambda nb0=nb0: proj_chunk(t_kTf[m], kw, m, nb0, False))
                units.append(lambda: fin_half(t_kTf[m], 0))
                units.append(lambda: (fin_half(t_kTf[m], 512), fin_det(t_kTf[m])))
                for nb0 in range(0, SP, SQB):
                    units.append(lambda nb0=nb0: proj_chunk(t_qTf[m], qwl, m, nb0, True))
                units.append(lambda: fin_half(t_qTf[m], 0))
                units.append(lambda: (fin_half(t_qTf[m], 512), fin_det(t_qTf[m])))
                return units

            # ---- prelude: tile 0 + all of v ----
            for u in tile_work(0):
                u()
            for mt in range(NT):
                v_chunk(mt, 0)
                v_chunk(mt, SQB)
            oc3 = t_onescol[:, :].rearrange("p (h o) -> p h o", o=1)
            for mt in range(NT):
                v3 = t_v[mt][:, :].rearrange("p (h j) -> p h j", j=65)
                nc.vector.tensor_copy(v3[:, :, HD:65], oc3)
            for i in range(NE):
                nc.gpsimd.memset(t_e[i][:, S:SP], 0.0)
            vst.close()  # v weights no longer needed

            # remaining projection work, popped between attention steps
            fillers = []
            for m in range(1, KT):
                fillers.extend(tile_work(m))
            fillers.reverse()  # pop() from the front of the logical order
            done_tile = [True] + [False] * (KT - 1)

            def pop_filler(n=1):
                for _ in range(n):
                    if fillers:
                        fillers.pop()()

            def drain_to_tile(mi):
                """Ensure kT/qT tile mi is fully emitted before use."""
                need = (KT - 1 - mi) * 10  # 10 units per remaining tile
                while len(fillers) > need:
                    fillers.pop()()
                done_tile[mi] = True

            # ---- attention ----
            ps_scB = None
            scB_stack = ExitStack()
            ei = 0
            step = 0
            pend_ctx = None  # (h, skt, e_tile) trailing ctx matmuls

            def emit_ctx(h, skt, e, ctx3):
                for mt in range(NT):
                    nc.tensor.matmul(ctx3[:, mt, 0:65],
                                     e[:, mt * 128:(mt + 1) * 128],
                                     t_v[skt][:, h * 65:h * 65 + 65],
                                     start=(skt == 0), stop=(skt == NT - 1))

            for h in range(H):
                hp, hr = h // 2, (h % 2) * 64
                if not done_tile[hp]:
                    drain_to_tile(hp)
                if not fillers and ps_scB is None:
                    # projections done: free their psum, open the second
                    # score buffer for double-buffered (whole-exp) mode
                    pB.close()
                    ps_scB = scB_stack.enter_context(
                        tc.tile_pool(name="ps_scB", bufs=1, space="PSUM"))
                p_ctx = ps_ctx.tile([128, NT * CS], F32, tag="ctx")
                ctx3 = p_ctx[:, :].rearrange("p (m w) -> p m w", w=CS)
                for skt in range(NT):
                    if ps_scB is not None and (step % 2 == 1):
                        sc = ps_scB.tile([128, SP], F32, tag="scB")
                    else:
                        sc = ps_scA.tile([128, SP], F32, tag="scA")
                    for c0, cw in ((0, 512), (512, 512), (1024, 128)):
                        nc.tensor.matmul(sc[:, c0:c0 + cw],
                                         t_kTf[hp][hr:hr + 64, skt * 128:(skt + 1) * 128],
                                         t_qTf[hp][hr:hr + 64, c0:c0 + cw],
                                         start=True, stop=True)
                    e = t_e[ei % NE]
                    ei += 1
                    if ps_scB is None:
                        # split exp: the [0:512] read completes early so the
                        # next step's first score chunk can reuse the psum
                        nc.scalar.activation(e[:, 0:512], sc[:, 0:512], AF.Exp,
                                             bias=0.0, scale=1.0)
                        nc.scalar.activation(e[:, 512:S], sc[:, 512:S], AF.Exp,
                                             bias=0.0, scale=1.0)
                    else:
                        nc.scalar.activation(e[:, 0:S], sc[:, 0:S], AF.Exp,
                                             bias=0.0, scale=1.0)
                    nc.vector.tensor_mul(e[:, 0:S], e[:, 0:S], t_EG[skt][:, 0:S])
                    if ps_scB is None and step % cfg.FILLER_EVERY == 0:
                        pop_filler()
                    if pend_ctx is not None:
                        emit_ctx(*pend_ctx)
                    pend_ctx = (h, skt, e, ctx3)
                    step += 1
                # flush the trailing ctx of skt=8 before normalization
                emit_ctx(*pend_ctx)
                pend_ctx = None
                rec = t_rec[h % 2]
                r3 = rec[:, :].rearrange("p (m o) -> p m o", o=1)
                nc.vector.reciprocal(r3[:, :, :], ctx3[:, :, 64:65])
                for mt in range(NT):
                    nc.gpsimd.tensor_scalar_mul(
                        t_ctx[mt][:, h * HD:(h + 1) * HD],
                        ctx3[:, mt, 0:HD], rec[:, mt:mt + 1])
                if h % 2 == 1:
                    for mt in range(NT):
                        nc.sync.dma_start_transpose(
                            t_ctxT[hp][:, mt * 128:(mt + 1) * 128],
                            t_ctx[mt][:, hp * 128:(hp + 1) * 128])
            scB_stack.close()
            apsum.close()  # free attention psum before the tail pool opens

            # ================== tail: output projection =====================
            with ExitStack() as p4:
                sb4 = p4.enter_context(tc.tile_pool(name="sb4", bufs=1))
                ps_o = p4.enter_context(tc.tile_pool(name="ps_o", bufs=3, space="PSUM"))
                for mt in range(NT):
                    t_out = sb4.tile([128, cfg.D], F32, tag="out", bufs=3)
                    p_o = ps_o.tile([128, cfg.D], F32, tag="po")
                    for n0 in range(0, cfg.D, 512):
                        nw = min(512, cfg.D - n0)
                        for k in range(KT):
                            nc.tensor.matmul(p_o[:, n0:n0 + nw],
                                             t_ctxT[k][:, mt * 128:(mt + 1) * 128],
                                             t_ow[k][:, n0:n0 + nw],
                                             start=(k == 0), stop=(k == KT - 1))
                    nc.vector.tensor_add(t_out[:], p_o[:], t_ob[:])
                    nc.sync.dma_start(d_out[mt * 128:(mt + 1) * 128, :], t_out[:])

    nc.compile()
    return nc


# ---------------- host-side prep + dispatch --------------------------------


def _host_prep(cfg: CFG, hidden_states, q_w, q_b, k_w, v_w, v_b, o_w, o_b,
               cos, sin, ph, pw, gate):
    KT, SP, H, HD = cfg.KT, cfg.SP, cfg.H, cfg.HD
    D, S = cfg.D, cfg.S
    DET = cfg.DET_END - cfg.DET_START
    half = HD // 2
    scale = HD ** -0.5

    shared = {}
    shared["qwT"] = round_f32r(q_w.T * scale)
    shared["kwT"] = round_f32r(k_w.T)
    vwT = np.zeros(((KT + 1) * 128, D), np.float32)
    vwT[:D] = v_w.T
    vwT[D] = v_b
    shared["vwT"] = round_f32r(vwT)
    shared["owT"] = round_f32r(o_w.T)
    # multiplicative gate: exp(gate^T), pad keys killed via exp(GATE_NEG)
    gateT = np.full((SP, SP), cfg.GATE_NEG, np.float32)
    gateT[:S, :S] = gate[0, 0].T
    shared["EG"] = to_bf16(np.exp(gateT))
    # q bias pre-scaled, laid out [128, KT]
    qb = (q_b.astype(np.float32) * scale).reshape(KT, 128).T
    shared["qb"] = np.ascontiguousarray(qb)
    shared["ob"] = np.broadcast_to(o_b.astype(np.float32)[None, :], (128, D)).copy()
    # rope tables: [128, RL] = two stacked head-copies of cos/sin transposed
    cosT = cos.T.astype(np.float32)                       # [HD, RL]
    sinT = sin.T.astype(np.float32)
    shared["cosT2"] = np.vstack([cosT, cosT]).astype(np.float32)
    shared["sinT2"] = np.vstack([sinT, sinT]).astype(np.float32)
    # rotation matrix R (rotate_half along the hd partition dim), applied as
    # R @ x via lhsT = R.T; R spans two stacked heads per 128-partition tile
    R = np.zeros((128, 128), np.float32)
    for blk in range(2):
        o = blk * HD
        for j in range(half):
            R[o + j, o + half + j] = -1.0
            R[o + half + j, o + j] = 1.0
    shared["rotT"] = round_f32r(R.T)
    shared["ones"] = round_f32r(np.ones((1, 128), np.float32))
    shared["onescol"] = to_bf16(np.ones((128, H), np.float32))
    maska = np.zeros((1, 128), np.float32)
    maskb = np.zeros((1, 128), np.float32)
    for p in range(128):
        if (p % HD) < half:
            maska[0, p] = 1.0
        else:
            maskb[0, p] = 1.0
    shared["masks"] = round_f32r(np.concatenate([maska, maskb], axis=1))
    shared["ph"] = ph.astype(np.float32).reshape(1, DET)
    shared["pw"] = pw.astype(np.float32).reshape(1, DET)

    in_maps = []
    for c in range(cfg.N_CORES):
        hsT = np.zeros((D, SP), np.float32)
        hsT[:, :S] = hidden_states[c].T
        m = dict(shared)
        m["hsT"] = to_bf16(hsT)
        in_maps.append(m)
    return in_maps


_NC_CACHE = {}


def kernel(hidden_states, q_w, q_b, k_w, v_w, v_b, o_w, o_b,
           cos, sin, ph, pw, gate,
           rope_start=5, rope_end=1029, det_start=1029, det_end=1129):
    cfg = CFG()
    in_maps = _host_prep(cfg, np.asarray(hidden_states, np.float32),
                         np.asarray(q_w, np.float32), np.asarray(q_b, np.float32),
                         np.asarray(k_w, np.float32), np.asarray(v_w, np.float32),
                         np.asarray(v_b, np.float32), np.asarray(o_w, np.float32),
                         np.asarray(o_b, np.float32), np.asarray(cos, np.float32),
                         np.asarray(sin, np.float32), np.asarray(ph, np.float32),
                         np.asarray(pw, np.float32), np.asarray(gate, np.float32))
    if "nc" not in _NC_CACHE:
        _NC_CACHE["nc"] = build_nc(cfg)
    nc = _NC_CACHE["nc"]
    res = run_bass_kernel_spmd(nc, in_maps, list(range(cfg.N_CORES)))
    out = np.stack([res.results[c]["out"][:cfg.S] for c in range(cfg.N_CORES)])
    return out.astype(np.float32)


# revision 17
# speedup vs baseline: 1.2334x; 1.2334x over previous
"""DINOv3 ViT attention (RoPE + det-temp scaling + additive gate) on 8 TRN2 cores.

Sharding: pure data-parallel over batch (B=8 -> 1 batch element per core).
Weights / gate / rope tables replicated. No collectives.

v3 design (engine budget per the TimelineSim cost model):
  - gate folded multiplicatively: EG = exp(gate^T) precomputed on host (bf16);
    after the ACT exp of raw scores, one DVE bf16 multiply applies it
    (replaces the per-head identity-matmul gate copies on PE).
  - scores per (head, skt): qT bf16 (moving operand -> 1 cyc/col at any
    width), kT bf16 (stationary), full-sq psum [128, 1152] in 3 chunks.
  - exp on ACT in 1-2 wide ops per (head, skt) (amortizes the ~185ns per-op
    access-latency penalty).
  - ctx flipped: out[sq_tile, 65] = e'^T @ v  (lhsT = e' bf16), N=65 per
    matmul -> half the PE columns of the [hd, sq] orientation; the v ones
    column lands the softmax denominator as a per-partition column, so
    normalization is one strided DVE reciprocal + 9 GpSimd scalar muls.
  - ctx [sq, hd] bf16 -> ctxT [hd, sq] via DMA XBAR transposes (idle engine).
  - phase merge: attention for head pair p starts as soon as kT/qT tile p is
    finalized; remaining projection/RoPE work is emitted as PE filler between
    attention steps.  While projection psum pools are open the score psum is
    single-buffered (8-bank budget) and the exp is split in two so the
    next-step score matmuls unblock early; once projections finish, a second
    score psum opens and exps run whole.
"""
import numpy as np
from contextlib import ExitStack

import ml_dtypes
import concourse.bacc as bacc
import concourse.mybir as mybir
import concourse.tile as tile
from concourse.bass_utils import run_bass_kernel_spmd

F32 = mybir.dt.float32
F32R = mybir.dt.float32r
BF16 = mybir.dt.bfloat16
AF = mybir.ActivationFunctionType

# ---------------- problem config (hardcoded per harness contract) ------------


class CFG:
    B = 8
    S = 1129
    SP = 1152            # padded S (9 * 128)
    D = 768
    H = 12
    HD = 64
    ROPE_START = 5
    ROPE_END = 1029
    DET_START = 1029
    DET_END = 1129
    P_SCALE = 2.0
    N_CORES = 8
    SQB = 384            # projection psum chunk
    GATE_NEG = -30.0     # gate value for pad keys: exp(-30) ~ 9e-14
    CTX_STRIDE = 74      # ctx psum window stride (65-wide windows, no
                         # 512-col psum bank crossings for 9 windows)
    FILLER_EVERY = 2     # pop one projection filler every N attention steps

    @property
    def KT(self):
        return self.D // 128          # dout/din 128-tiles (6)

    @property
    def NT(self):
        return self.SP // 128         # s 128-tiles (9)

    @property
    def ROPE_LEN(self):
        return self.ROPE_END - self.ROPE_START


def round_f32r(x: np.ndarray) -> np.ndarray:
    """Round fp32 to the fp32r format (11 mantissa bits, RNE)."""
    b = np.ascontiguousarray(x, dtype=np.float32).view(np.uint32)
    low = b & np.uint32(0xFFF)
    b = b & np.uint32(0xFFFFF000)
    rnd = (low > 0x800) | ((low == 0x800) & (((b >> 12) & 1) != 0))
    b = b + (rnd.astype(np.uint32) << 12)
    return b.view(np.float32)


def to_bf16(x: np.ndarray) -> np.ndarray:
    return np.ascontiguousarray(x, dtype=np.float32).astype(ml_dtypes.bfloat16)


# ---------------- device program ------------------------------------------


def build_nc(cfg: CFG):
    nc = bacc.Bacc(trn_type="TRN2", target_bir_lowering=False, debug=False)
    KT, NT, SQB, SP = cfg.KT, cfg.NT, cfg.SQB, cfg.SP
    H, HD = cfg.H, cfg.HD
    RS, RE, DS, DE = cfg.ROPE_START, cfg.ROPE_END, cfg.DET_START, cfg.DET_END
    RL = cfg.ROPE_LEN
    DET = DE - DS
    S = cfg.S
    CS = cfg.CTX_STRIDE

    # ---- dram parameters (per core) ----
    d_hsT = nc.dram_tensor("hsT", [cfg.D, SP], BF16, kind="ExternalInput").ap()
    d_qwT = nc.dram_tensor("qwT", [cfg.D, cfg.D], F32R, kind="ExternalInput").ap()
    d_kwT = nc.dram_tensor("kwT", [cfg.D, cfg.D], F32R, kind="ExternalInput").ap()
    d_vwT = nc.dram_tensor("vwT", [(KT + 1) * 128, cfg.D], F32R, kind="ExternalInput").ap()
    d_owT = nc.dram_tensor("owT", [cfg.D, cfg.D], F32R, kind="ExternalInput").ap()
    d_EG = nc.dram_tensor("EG", [SP, SP], BF16, kind="ExternalInput").ap()
    d_qb = nc.dram_tensor("qb", [128, KT], F32, kind="ExternalInput").ap()
    d_ob = nc.dram_tensor("ob", [128, cfg.D], F32, kind="ExternalInput").ap()
    d_cosT2 = nc.dram_tensor("cosT2", [128, RL], F32, kind="ExternalInput").ap()
    d_sinT2 = nc.dram_tensor("sinT2", [128, RL], F32, kind="ExternalInput").ap()
    d_rotT = nc.dram_tensor("rotT", [128, 128], F32R, kind="ExternalInput").ap()
    d_ones = nc.dram_tensor("ones", [1, 128], F32R, kind="ExternalInput").ap()
    d_onescol = nc.dram_tensor("onescol", [128, H], BF16, kind="ExternalInput").ap()
    d_masks = nc.dram_tensor("masks", [1, 256], F32R, kind="ExternalInput").ap()
    d_ph = nc.dram_tensor("ph", [1, DET], F32, kind="ExternalInput").ap()
    d_pw = nc.dram_tensor("pw", [1, DET], F32, kind="ExternalInput").ap()
    d_out = nc.dram_tensor("out", [SP, cfg.D], F32, kind="ExternalOutput").ap()

    with tile.TileContext(nc) as tc, ExitStack() as gctx:
        # ---------------- global sbuf (spans the whole kernel) --------------
        gsb = gctx.enter_context(tc.tile_pool(name="gsb", bufs=1))

        t_ones = gsb.tile([1, 128], F32R, tag="ones")
        nc.sync.dma_start(t_ones[:], d_ones[:, :])

        # persistent activations
        t_qTf = [gsb.tile([128, SP], BF16, tag=f"qTf{m}", name=f"qTf{m}") for m in range(KT)]
        t_kTf = [gsb.tile([128, SP], BF16, tag=f"kTf{m}", name=f"kTf{m}") for m in range(KT)]
        t_v = [gsb.tile([128, H * 65], BF16, tag=f"v{t}", name=f"v{t}") for t in range(NT)]
        t_EG = [gsb.tile([128, SP], BF16, tag=f"eg{t}", name=f"eg{t}") for t in range(NT)]
        NE = 4
        t_e = [gsb.tile([128, SP], BF16, tag=f"e{i}", name=f"e{i}") for i in range(NE)]
        t_ctx = [gsb.tile([128, cfg.D], BF16, tag=f"ctx{t}", name=f"ctx{t}") for t in range(NT)]
        t_ctxT = [gsb.tile([128, SP], BF16, tag=f"ctxT{k}", name=f"ctxT{k}") for k in range(KT)]
        t_rec = [gsb.tile([128, NT], F32, tag=f"rec{i}", name=f"rec{i}") for i in range(2)]
        t_tem = gsb.tile([128, DET], F32, tag="tem")

        # ---- det temperature pattern (standalone psum stack) ----
        with ExitStack() as ptem:
            sbt = ptem.enter_context(tc.tile_pool(name="sbt", bufs=1))
            ps_tem = ptem.enter_context(tc.tile_pool(name="ps_tem", bufs=1, space="PSUM"))
            t_ms = sbt.tile([1, 256], F32R, tag="ms")
            nc.sync.dma_start(t_ms[:], d_masks[:, :])
            t_ph = sbt.tile([1, DET], F32, tag="ph")
            nc.sync.dma_start(t_ph[:], d_ph[:, :])
            t_pw = sbt.tile([1, DET], F32, tag="pw")
            nc.sync.dma_start(t_pw[:], d_pw[:, :])
            t_eh = sbt.tile([1, DET], F32R, tag="eh")
            nc.scalar.activation(t_eh[:], t_ph[:], AF.Exp, bias=0.0, scale=cfg.P_SCALE)
            t_ew = sbt.tile([1, DET], F32R, tag="ew")
            nc.scalar.activation(t_ew[:], t_pw[:], AF.Exp, bias=0.0, scale=cfg.P_SCALE)
            p_tem = ps_tem.tile([128, DET], F32, tag="tem")
            nc.tensor.matmul(p_tem[:], t_ms[0:1, 0:128], t_eh[:], start=True, stop=False)
            nc.tensor.matmul(p_tem[:], t_ms[0:1, 128:256], t_ew[:], start=False, stop=True)
            nc.vector.tensor_copy(t_tem[:], p_tem[:])

        # ==================== merged projections + attention ================
        # psum layout (8 banks, scores double-buffered while projection
        # pools are open):
        #   mA, mB: [128,1024] score mains (2 banks each)
        #   X:      [128,512] shared bank: 3 rotating 105-col score sq-tails
        #           + ctx windows for sq tiles 7,8 (at 320, 393)
        #   c7:     [128,512] ctx windows for sq tiles 0..6 (stride 73)
        #   scratch:[128,512] x2 projection chunks / rope psum
        TW = S - 1024                    # 105: sq tail width
        XC = 320                         # ctx78 base offset inside X
        CSW = 73                         # ctx window stride
        with ExitStack() as att:
            sb3 = att.enter_context(tc.tile_pool(name="sb3", bufs=1))
            t_ob = sb3.tile([128, cfg.D], F32, tag="ob")
            t_ow = [sb3.tile([128, cfg.D], F32R, tag=f"ow{k}", name=f"ow{k}")
                    for k in range(KT)]
            apsum = att.enter_context(ExitStack())
            ps_mA = apsum.enter_context(tc.tile_pool(name="ps_mA", bufs=1, space="PSUM"))
            ps_mB = apsum.enter_context(tc.tile_pool(name="ps_mB", bufs=1, space="PSUM"))
            ps_X = apsum.enter_context(tc.tile_pool(name="ps_X", bufs=1, space="PSUM"))
            ps_c7 = apsum.enter_context(tc.tile_pool(name="ps_c7", bufs=1, space="PSUM"))

            pB = ExitStack()
            sb1 = pB.enter_context(tc.tile_pool(name="sb1", bufs=1))
            wsb = pB.enter_context(tc.tile_pool(name="wsb", bufs=1))
            ps_s = pB.enter_context(tc.tile_pool(name="ps_s", bufs=2, space="PSUM"))

            # ---- loads: hsT/vw first (v-projection leads), then kw, qw ----
            vst = ExitStack()
            vsb = vst.enter_context(tc.tile_pool(name="vsb", bufs=1))

            t_hsT = []
            vwl = []
            for k in range(KT):
                t = sb1.tile([128, SP], BF16, tag=f"hsT{k}", name=f"hsT{k}")
                eng = nc.sync if k % 2 == 0 else nc.scalar
                eng.dma_start(t[:], d_hsT[k * 128:(k + 1) * 128, :])
                t_hsT.append(t)
                w = vsb.tile([128, cfg.D], F32R, tag=f"vw{k}", name=f"vw{k}")
                weng = nc.scalar if k % 2 == 0 else nc.sync
                weng.dma_start(w[:], d_vwT[k * 128:(k + 1) * 128, :])
                vwl.append(w)
            t_vb = sb1.tile([1, cfg.D], F32R, tag="vb")
            nc.sync.dma_start(t_vb[:], d_vwT[cfg.D:cfg.D + 1, :])
            t_onescol = sb1.tile([128, H], BF16, tag="onescol")
            nc.sync.dma_start(t_onescol[:], d_onescol[:, :])
            kw = []
            for k in range(KT):
                w = wsb.tile([128, cfg.D], F32R, tag=f"w{k}", name=f"kw{k}")
                (nc.scalar if k % 2 == 0 else nc.sync).dma_start(
                    w[:], d_kwT[k * 128:(k + 1) * 128, :])
                kw.append(w)
            t_cos = sb1.tile([128, RL], F32, tag="cos")
            nc.scalar.dma_start(t_cos[:], d_cosT2[:, :])
            t_sin = sb1.tile([128, RL], F32, tag="sin")
            nc.scalar.dma_start(t_sin[:], d_sinT2[:, :])
            t_rotT = sb1.tile([128, 128], F32R, tag="rotT")
            nc.sync.dma_start(t_rotT[:], d_rotT[:, :])
            t_qb = sb1.tile([128, KT], F32, tag="qb")
            nc.sync.dma_start(t_qb[:], d_qb[:, :])
            qwl = []
            for k in range(KT):
                w = sb1.tile([128, cfg.D], F32R, tag=f"qw{k}", name=f"qw{k}")
                (nc.sync if k % 2 == 0 else nc.scalar).dma_start(
                    w[:], d_qwT[k * 128:(k + 1) * 128, :])
                qwl.append(w)
            for t in range(NT):
                nc.scalar.dma_start(t_EG[t][:], d_EG[t * 128:(t + 1) * 128, :])
            nc.sync.dma_start(t_ob[:], d_ob[:, :])
            for k in range(KT):
                nc.sync.dma_start(t_ow[k][:], d_owT[k * 128:(k + 1) * 128, :])

            # ---- projection / finalize emitters (scratch psum) ----
            def fin_half(dst, c0):
                """RoPE on dst[:, RS+c0 : RS+c0+512] in place."""
                p_rot = ps_s.tile([128, 512], F32, tag="qp", name="rot")
                nc.tensor.matmul(p_rot[:], t_rotT[:], dst[:, RS + c0:RS + c0 + 512],
                                 start=True, stop=True)
                tmp1 = sb1.tile([128, 512], F32, tag="tmp1", bufs=2)
                nc.vector.tensor_mul(tmp1[:], p_rot[:], t_sin[:, c0:c0 + 512])
                nc.gpsimd.tensor_mul(dst[:, RS + c0:RS + c0 + 512],
                                     dst[:, RS + c0:RS + c0 + 512],
                                     t_cos[:, c0:c0 + 512])
                nc.vector.tensor_add(dst[:, RS + c0:RS + c0 + 512],
                                     dst[:, RS + c0:RS + c0 + 512], tmp1[:])

            def fin_det(dst):
                nc.gpsimd.tensor_mul(dst[:, DS:DE], dst[:, DS:DE], t_tem[:])

            def proj_chunk(dst_tile, wlist, m, nb0, is_q):
                p = ps_s.tile([128, 512], F32, tag="qp")
                for k in range(KT):
                    nc.tensor.matmul(p[:, 0:SQB], wlist[k][:, m * 128:(m + 1) * 128],
                                     t_hsT[k][:, nb0:nb0 + SQB],
                                     start=(k == 0), stop=(k == KT - 1))
                if is_q:
                    nc.vector.tensor_scalar_add(dst_tile[:, nb0:nb0 + SQB],
                                                p[:, 0:SQB], t_qb[:, m:m + 1])
                else:
                    nc.scalar.copy(dst_tile[:, nb0:nb0 + SQB], p[:, 0:SQB])

            def v_chunk(mt, n0):
                p = ps_s.tile([128, 512], F32, tag="qp", name=f"vp{mt}_{n0}")
                nc.tensor.matmul(p[:, 0:SQB], t_ones[0:1, :], t_vb[0:1, n0:n0 + SQB],
                                 start=True, stop=False)
                for k in range(KT):
                    nc.tensor.matmul(p[:, 0:SQB], t_hsT[k][:, mt * 128:(mt + 1) * 128],
                                     vwl[k][:, n0:n0 + SQB],
                                     start=False, stop=(k == KT - 1))
                nh = SQB // HD  # heads covered by this chunk (6)
                h0 = n0 // HD
                vin = p[:, 0:SQB].rearrange("p (h j) -> p h j", h=nh)
                v3 = t_v[mt][:, h0 * 65:(h0 + nh) * 65].rearrange(
                    "p (h j) -> p h j", j=65)
                nc.scalar.activation(v3[:, :, 0:HD], vin, AF.Identity,
                                     bias=0.0, scale=1.0)

            def tile_units(m):
                """(tile_idx, closure) units building kT[m], qT[m] + finalize."""
                units = []
                for nb0 in range(0, SP, SQB):
                    units.append((m, lambda m=m, nb0=nb0: proj_chunk(t_kTf[m], kw, m, nb0, False)))
                units.append((m, lambda m=m: fin_half(t_kTf[m], 0)))
                units.append((m, lambda m=m: (fin_half(t_kTf[m], 512), fin_det(t_kTf[m]))))
                for nb0 in range(0, SP, SQB):
                    units.append((m, lambda m=m, nb0=nb0: proj_chunk(t_qTf[m], qwl, m, nb0, True)))
                units.append((m, lambda m=m: fin_half(t_qTf[m], 0)))
                units.append((m, lambda m=m: (fin_half(t_qTf[m], 512), fin_det(t_qTf[m]))))
                return units

            # ---- prelude: all of v, then kT/qT tile 0 ----
            for mt in range(NT):
                v_chunk(mt, 0)
                v_chunk(mt, SQB)
            oc3 = t_onescol[:, :].rearrange("p (h o) -> p h o", o=1)
            for mt in range(NT):
                v3 = t_v[mt][:, :].rearrange("p (h j) -> p h j", j=65)
                nc.vector.tensor_copy(v3[:, :, HD:65], oc3)
            for i in range(NE):
                nc.gpsimd.memset(t_e[i][:, S:SP], 0.0)
            vst.close()  # v weights no longer needed
            for _, u in tile_units(0):
                u()

            # remaining projection work, popped between attention steps
            fillers = []
            for m in range(1, KT):
                fillers.extend(tile_units(m))
            fillers.reverse()  # pop from the end = logical order

            def pop_filler():
                if fillers:
                    fillers.pop()[1]()

            def drain_to_tile(mi):
                while fillers and any(i <= mi for i, _ in fillers):
                    fillers.pop()[1]()

            # ---- attention ----
            xt = ps_X.tile([128, 512], F32, tag="X")
            ei = 0
            step = 0
            pend_ctx = None
            pB_open = True

            def emit_ctx(h, skt, e, c73):
                for mt in range(7):
                    nc.tensor.matmul(c73[:, mt, 0:65],
                                     e[:, mt * 128:(mt + 1) * 128],
                                     t_v[skt][:, h * 65:h * 65 + 65],
                                     start=(skt == 0), stop=(skt == NT - 1))
                for mt in (7, 8):
                    o = XC + CSW * (mt - 7)
                    nc.tensor.matmul(xt[:, o:o + 65],
                                     e[:, mt * 128:(mt + 1) * 128],
                                     t_v[skt][:, h * 65:h * 65 + 65],
                                     start=(skt == 0), stop=(skt == NT - 1))

            for h in range(H):
                hp, hr = h // 2, (h % 2) * 64
                drain_to_tile(hp)
                if not fillers and pB_open:
                    pB.close()
                    pB_open = False
                p_c7 = ps_c7.tile([128, 512], F32, tag="c7")
                c73 = p_c7[:, 0:CSW * 7].rearrange("p (m w) -> p m w", w=CSW)
                for skt in range(NT):
                    sc = (ps_mA if step % 2 == 0 else ps_mB).tile(
                        [128, 1024], F32, tag="m")
                    ti = step % 3
                    for c0, cw in ((0, 512), (512, 512)):
                        nc.tensor.matmul(sc[:, c0:c0 + cw],
                                         t_kTf[hp][hr:hr + 64, skt * 128:(skt + 1) * 128],
                                         t_qTf[hp][hr:hr + 64, c0:c0 + cw],
                                         start=True, stop=True)
                    nc.tensor.matmul(xt[:, ti * TW:(ti + 1) * TW],
                                     t_kTf[hp][hr:hr + 64, skt * 128:(skt + 1) * 128],
                                     t_qTf[hp][hr:hr + 64, 1024:S],
                                     start=True, stop=True)
                    e = t_e[ei % NE]
                    ei += 1
                    nc.scalar.activation(e[:, 0:1024], sc[:, 0:1024], AF.Exp,
                                         bias=0.0, scale=1.0)
                    nc.scalar.activation(e[:, 1024:S], xt[:, ti * TW:(ti + 1) * TW],
                                         AF.Exp, bias=0.0, scale=1.0)
                    nc.vector.tensor_mul(e[:, 0:S], e[:, 0:S], t_EG[skt][:, 0:S])
                    if pB_open and step % cfg.FILLER_EVERY == 0:
                        pop_filler()
                    if pend_ctx is not None:
                        emit_ctx(*pend_ctx)
                    pend_ctx = (h, skt, e, c73)
                    step += 1
                emit_ctx(*pend_ctx)
                pend_ctx = None
                rec = t_rec[h % 2]
                r3 = rec[:, 0:7].rearrange("p (m o) -> p m o", o=1)
                nc.vector.reciprocal(r3[:, :, :], c73[:, :, 64:65])
                nc.vector.reciprocal(rec[:, 7:8], xt[:, XC + 64:XC + 65])
                nc.vector.reciprocal(rec[:, 8:9], xt[:, XC + CSW + 64:XC + CSW + 65])
                for mt in range(7):
                    nc.gpsimd.tensor_scalar_mul(
                        t_ctx[mt][:, h * HD:(h + 1) * HD],
                        c73[:, mt, 0:HD], rec[:, mt:mt + 1])
                for mt in (7, 8):
                    o = XC + CSW * (mt - 7)
                    nc.gpsimd.tensor_scalar_mul(
                        t_ctx[mt][:, h * HD:(h + 1) * HD],
                        xt[:, o:o + HD], rec[:, mt:mt + 1])
                if h % 2 == 1:
                    for mt in range(NT):
                        nc.sync.dma_start_transpose(
                            t_ctxT[hp][:, mt * 128:(mt + 1) * 128],
                            t_ctx[mt][:, hp * 128:(hp + 1) * 128])
            if pB_open:
                pB.close()
            apsum.close()  # free attention psum before the tail pool opens

            # ================== tail: output projection =====================
            with ExitStack() as p4:
                sb4 = p4.enter_context(tc.tile_pool(name="sb4", bufs=1))
                ps_o = p4.enter_context(tc.tile_pool(name="ps_o", bufs=3, space="PSUM"))
                for mt in range(NT):
                    t_out = sb4.tile([128, cfg.D], F32, tag="out", bufs=3)
                    p_o = ps_o.tile([128, cfg.D], F32, tag="po")
                    for n0 in range(0, cfg.D, 512):
                        nw = min(512, cfg.D - n0)
                        for k in range(KT):
                            nc.tensor.matmul(p_o[:, n0:n0 + nw],
                                             t_ctxT[k][:, mt * 128:(mt + 1) * 128],
                                             t_ow[k][:, n0:n0 + nw],
                                             start=(k == 0), stop=(k == KT - 1))
                    nc.vector.tensor_add(t_out[:], p_o[:], t_ob[:])
                    nc.sync.dma_start(d_out[mt * 128:(mt + 1) * 128, :], t_out[:])

    nc.compile()
    return nc


# ---------------- host-side prep + dispatch --------------------------------


def _host_prep(cfg: CFG, hidden_states, q_w, q_b, k_w, v_w, v_b, o_w, o_b,
               cos, sin, ph, pw, gate):
    KT, SP, H, HD = cfg.KT, cfg.SP, cfg.H, cfg.HD
    D, S = cfg.D, cfg.S
    DET = cfg.DET_END - cfg.DET_START
    half = HD // 2
    scale = HD ** -0.5

    shared = {}
    shared["qwT"] = round_f32r(q_w.T * scale)
    shared["kwT"] = round_f32r(k_w.T)
    vwT = np.zeros(((KT + 1) * 128, D), np.float32)
    vwT[:D] = v_w.T
    vwT[D] = v_b
    shared["vwT"] = round_f32r(vwT)
    shared["owT"] = round_f32r(o_w.T)
    # multiplicative gate: exp(gate^T), pad keys killed via exp(GATE_NEG)
    gateT = np.full((SP, SP), cfg.GATE_NEG, np.float32)
    gateT[:S, :S] = gate[0, 0].T
    shared["EG"] = to_bf16(np.exp(gateT))
    # q bias pre-scaled, laid out [128, KT]
    qb = (q_b.astype(np.float32) * scale).reshape(KT, 128).T
    shared["qb"] = np.ascontiguousarray(qb)
    shared["ob"] = np.broadcast_to(o_b.astype(np.float32)[None, :], (128, D)).copy()
    # rope tables: [128, RL] = two stacked head-copies of cos/sin transposed
    cosT = cos.T.astype(np.float32)                       # [HD, RL]
    sinT = sin.T.astype(np.float32)
    shared["cosT2"] = np.vstack([cosT, cosT]).astype(np.float32)
    shared["sinT2"] = np.vstack([sinT, sinT]).astype(np.float32)
    # rotation matrix R (rotate_half along the hd partition dim), applied as
    # R @ x via lhsT = R.T; R spans two stacked heads per 128-partition tile
    R = np.zeros((128, 128), np.float32)
    for blk in range(2):
        o = blk * HD
        for j in range(half):
            R[o + j, o + half + j] = -1.0
            R[o + half + j, o + j] = 1.0
    shared["rotT"] = round_f32r(R.T)
    shared["ones"] = round_f32r(np.ones((1, 128), np.float32))
    shared["onescol"] = to_bf16(np.ones((128, H), np.float32))
    maska = np.zeros((1, 128), np.float32)
    maskb = np.zeros((1, 128), np.float32)
    for p in range(128):
        if (p % HD) < half:
            maska[0, p] = 1.0
        else:
            maskb[0, p] = 1.0
    shared["masks"] = round_f32r(np.concatenate([maska, maskb], axis=1))
    shared["ph"] = ph.astype(np.float32).reshape(1, DET)
    shared["pw"] = pw.astype(np.float32).reshape(1, DET)

    in_maps = []
    for c in range(cfg.N_CORES):
        hsT = np.zeros((D, SP), np.float32)
        hsT[:, :S] = hidden_states[c].T
        m = dict(shared)
        m["hsT"] = to_bf16(hsT)
        in_maps.append(m)
    return in_maps


_NC_CACHE = {}


def kernel(hidden_states, q_w, q_b, k_w, v_w, v_b, o_w, o_b,
           cos, sin, ph, pw, gate,
           rope_start=5, rope_end=1029, det_start=1029, det_end=1129):
    cfg = CFG()
    in_maps = _host_prep(cfg, np.asarray(hidden_states, np.float32),
                         np.asarray(q_w, np.float32), np.asarray(q_b, np.float32),
                         np.asarray(k_w, np.float32), np.asarray(v_w, np.float32),
                         np.asarray(v_b, np.float32), np.asarray(o_w, np.float32),
                         np.asarray(o_b, np.float32), np.asarray(cos, np.float32),
                         np.asarray(sin, np.float32), np.asarray(ph, np.float32),
                         np.asarray(pw, np.float32), np.asarray(gate, np.float32))
    if "nc" not in _NC_CACHE:
        _NC_CACHE["nc"] = build_nc(cfg)
    nc = _NC_CACHE["nc"]
    res = run_bass_kernel_spmd(nc, in_maps, list(range(cfg.N_CORES)))
    out = np.stack([res.results[c]["out"][:cfg.S] for c in range(cfg.N_CORES)])
    return out.astype(np.float32)


# revision 18
# speedup vs baseline: 1.3428x; 1.0887x over previous
"""DINOv3 ViT attention (RoPE + det-temp scaling + additive gate) on 8 TRN2 cores.

Sharding: pure data-parallel over batch (B=8 -> 1 batch element per core).
Weights / gate / rope tables replicated. No collectives.

v3 design (engine budget per the TimelineSim cost model):
  - gate folded multiplicatively: EG = exp(gate^T) precomputed on host (bf16);
    after the ACT exp of raw scores, one DVE bf16 multiply applies it
    (replaces the per-head identity-matmul gate copies on PE).
  - scores per (head, skt): qT bf16 (moving operand -> 1 cyc/col at any
    width), kT bf16 (stationary), full-sq psum [128, 1152] in 3 chunks.
  - exp on ACT in 1-2 wide ops per (head, skt) (amortizes the ~185ns per-op
    access-latency penalty).
  - ctx flipped: out[sq_tile, 65] = e'^T @ v  (lhsT = e' bf16), N=65 per
    matmul -> half the PE columns of the [hd, sq] orientation; the v ones
    column lands the softmax denominator as a per-partition column, so
    normalization is one strided DVE reciprocal + 9 GpSimd scalar muls.
  - ctx [sq, hd] bf16 -> ctxT [hd, sq] via DMA XBAR transposes (idle engine).
  - phase merge: attention for head pair p starts as soon as kT/qT tile p is
    finalized; remaining projection/RoPE work is emitted as PE filler between
    attention steps.  While projection psum pools are open the score psum is
    single-buffered (8-bank budget) and the exp is split in two so the
    next-step score matmuls unblock early; once projections finish, a second
    score psum opens and exps run whole.
"""
import numpy as np
from contextlib import ExitStack

import ml_dtypes
import concourse.bacc as bacc
import concourse.mybir as mybir
import concourse.tile as tile
from concourse.bass_utils import run_bass_kernel_spmd

F32 = mybir.dt.float32
F32R = mybir.dt.float32r
BF16 = mybir.dt.bfloat16
AF = mybir.ActivationFunctionType

# ---------------- problem config (hardcoded per harness contract) ------------


class CFG:
    B = 8
    S = 1129
    SP = 1152            # padded S (9 * 128)
    D = 768
    H = 12
    HD = 64
    ROPE_START = 5
    ROPE_END = 1029
    DET_START = 1029
    DET_END = 1129
    P_SCALE = 2.0
    N_CORES = 8
    SQB = 384            # projection psum chunk
    GATE_NEG = -30.0     # gate value for pad keys: exp(-30) ~ 9e-14
    CTX_STRIDE = 74      # ctx psum window stride (65-wide windows, no
                         # 512-col psum bank crossings for 9 windows)
    FILLER_EVERY = 2     # pop one projection filler every N attention steps

    @property
    def KT(self):
        return self.D // 128          # dout/din 128-tiles (6)

    @property
    def NT(self):
        return self.SP // 128         # s 128-tiles (9)

    @property
    def ROPE_LEN(self):
        return self.ROPE_END - self.ROPE_START


def round_f32r(x: np.ndarray) -> np.ndarray:
    """Round fp32 to the fp32r format (11 mantissa bits, RNE)."""
    b = np.ascontiguousarray(x, dtype=np.float32).view(np.uint32)
    low = b & np.uint32(0xFFF)
    b = b & np.uint32(0xFFFFF000)
    rnd = (low > 0x800) | ((low == 0x800) & (((b >> 12) & 1) != 0))
    b = b + (rnd.astype(np.uint32) << 12)
    return b.view(np.float32)


def to_bf16(x: np.ndarray) -> np.ndarray:
    return np.ascontiguousarray(x, dtype=np.float32).astype(ml_dtypes.bfloat16)


# ---------------- device program ------------------------------------------


def build_nc(cfg: CFG):
    nc = bacc.Bacc(trn_type="TRN2", target_bir_lowering=False, debug=False)
    KT, NT, SQB, SP = cfg.KT, cfg.NT, cfg.SQB, cfg.SP
    H, HD = cfg.H, cfg.HD
    RS, RE, DS, DE = cfg.ROPE_START, cfg.ROPE_END, cfg.DET_START, cfg.DET_END
    RL = cfg.ROPE_LEN
    DET = DE - DS
    S = cfg.S
    CS = cfg.CTX_STRIDE

    # ---- dram parameters (per core) ----
    d_hsT = nc.dram_tensor("hsT", [cfg.D, SP], BF16, kind="ExternalInput").ap()
    d_qwT = nc.dram_tensor("qwT", [cfg.D, cfg.D], F32R, kind="ExternalInput").ap()
    d_kwT = nc.dram_tensor("kwT", [cfg.D, cfg.D], F32R, kind="ExternalInput").ap()
    d_vwT = nc.dram_tensor("vwT", [(KT + 1) * 128, cfg.D], F32R, kind="ExternalInput").ap()
    d_owT = nc.dram_tensor("owT", [cfg.D, cfg.D], F32R, kind="ExternalInput").ap()
    d_EG = nc.dram_tensor("EG", [SP, SP], BF16, kind="ExternalInput").ap()
    d_qb = nc.dram_tensor("qb", [128, KT], F32, kind="ExternalInput").ap()
    d_ob = nc.dram_tensor("ob", [128, cfg.D], F32, kind="ExternalInput").ap()
    d_cosT2 = nc.dram_tensor("cosT2", [128, RL], F32, kind="ExternalInput").ap()
    d_sinT2 = nc.dram_tensor("sinT2", [128, RL], F32, kind="ExternalInput").ap()
    d_rotT = nc.dram_tensor("rotT", [128, 128], F32R, kind="ExternalInput").ap()
    d_ones = nc.dram_tensor("ones", [1, 128], F32R, kind="ExternalInput").ap()
    d_onescol = nc.dram_tensor("onescol", [128, H], BF16, kind="ExternalInput").ap()
    d_masks = nc.dram_tensor("masks", [1, 256], F32R, kind="ExternalInput").ap()
    d_ph = nc.dram_tensor("ph", [1, DET], F32, kind="ExternalInput").ap()
    d_pw = nc.dram_tensor("pw", [1, DET], F32, kind="ExternalInput").ap()
    d_out = nc.dram_tensor("out", [SP, cfg.D], F32, kind="ExternalOutput").ap()

    with tile.TileContext(nc) as tc, ExitStack() as gctx:
        # ---------------- global sbuf (spans the whole kernel) --------------
        gsb = gctx.enter_context(tc.tile_pool(name="gsb", bufs=1))

        t_ones = gsb.tile([1, 128], F32R, tag="ones")
        nc.sync.dma_start(t_ones[:], d_ones[:, :])

        # persistent activations
        t_qTf = [gsb.tile([128, SP], BF16, tag=f"qTf{m}", name=f"qTf{m}") for m in range(KT)]
        t_kTf = [gsb.tile([128, SP], BF16, tag=f"kTf{m}", name=f"kTf{m}") for m in range(KT)]
        t_v = [gsb.tile([128, H * 65], BF16, tag=f"v{t}", name=f"v{t}") for t in range(NT)]
        t_EG = [gsb.tile([128, SP], BF16, tag=f"eg{t}", name=f"eg{t}") for t in range(NT)]
        NE = 4
        t_e = [gsb.tile([128, SP], BF16, tag=f"e{i}", name=f"e{i}") for i in range(NE)]
        t_ctx = [gsb.tile([128, cfg.D], BF16, tag=f"ctx{t}", name=f"ctx{t}") for t in range(NT)]
        t_ctxT = [gsb.tile([128, SP], BF16, tag=f"ctxT{k}", name=f"ctxT{k}") for k in range(KT)]
        t_rec = [gsb.tile([128, NT], F32, tag=f"rec{i}", name=f"rec{i}") for i in range(2)]
        t_tem = gsb.tile([128, DET], F32, tag="tem")

        # ---- det temperature pattern (standalone psum stack) ----
        with ExitStack() as ptem:
            sbt = ptem.enter_context(tc.tile_pool(name="sbt", bufs=1))
            ps_tem = ptem.enter_context(tc.tile_pool(name="ps_tem", bufs=1, space="PSUM"))
            t_ms = sbt.tile([1, 256], F32R, tag="ms")
            nc.sync.dma_start(t_ms[:], d_masks[:, :])
            t_ph = sbt.tile([1, DET], F32, tag="ph")
            nc.sync.dma_start(t_ph[:], d_ph[:, :])
            t_pw = sbt.tile([1, DET], F32, tag="pw")
            nc.sync.dma_start(t_pw[:], d_pw[:, :])
            t_eh = sbt.tile([1, DET], F32R, tag="eh")
            nc.scalar.activation(t_eh[:], t_ph[:], AF.Exp, bias=0.0, scale=cfg.P_SCALE)
            t_ew = sbt.tile([1, DET], F32R, tag="ew")
            nc.scalar.activation(t_ew[:], t_pw[:], AF.Exp, bias=0.0, scale=cfg.P_SCALE)
            p_tem = ps_tem.tile([128, DET], F32, tag="tem")
            nc.tensor.matmul(p_tem[:], t_ms[0:1, 0:128], t_eh[:], start=True, stop=False)
            nc.tensor.matmul(p_tem[:], t_ms[0:1, 128:256], t_ew[:], start=False, stop=True)
            nc.vector.tensor_copy(t_tem[:], p_tem[:])

        # ==================== merged projections + attention ================
        # psum layout (8 banks, scores double-buffered while projection
        # pools are open):
        #   mA, mB: [128,1024] score mains (2 banks each)
        #   X:      [128,512] shared bank: 3 rotating 105-col score sq-tails
        #           + ctx windows for sq tiles 7,8 (at 320, 393)
        #   c7:     [128,512] ctx windows for sq tiles 0..6 (stride 73)
        #   scratch:[128,512] x2 projection chunks / rope psum
        TW = S - 1024                    # 105: sq tail width
        XC = 320                         # ctx78 base offset inside X
        CSW = 73                         # ctx window stride
        with ExitStack() as att:
            sb3 = att.enter_context(tc.tile_pool(name="sb3", bufs=1))
            t_ob = sb3.tile([128, cfg.D], F32, tag="ob")
            t_ow = [sb3.tile([128, cfg.D], F32R, tag=f"ow{k}", name=f"ow{k}")
                    for k in range(KT)]
            apsum = att.enter_context(ExitStack())
            ps_mA = apsum.enter_context(tc.tile_pool(name="ps_mA", bufs=1, space="PSUM"))
            ps_mB = apsum.enter_context(tc.tile_pool(name="ps_mB", bufs=1, space="PSUM"))
            ps_X = apsum.enter_context(tc.tile_pool(name="ps_X", bufs=1, space="PSUM"))
            ps_c7 = apsum.enter_context(tc.tile_pool(name="ps_c7", bufs=1, space="PSUM"))

            pB = ExitStack()
            sb1 = pB.enter_context(tc.tile_pool(name="sb1", bufs=1))
            wsb = pB.enter_context(tc.tile_pool(name="wsb", bufs=1))
            ps_s = pB.enter_context(tc.tile_pool(name="ps_s", bufs=2, space="PSUM"))

            # ---- loads: hsT/vw first (v-projection leads), then kw, qw ----
            vst = ExitStack()
            vsb = vst.enter_context(tc.tile_pool(name="vsb", bufs=1))

            t_hsT = []
            vwl = []
            for k in range(KT):
                t = sb1.tile([128, SP], BF16, tag=f"hsT{k}", name=f"hsT{k}")
                nc.sync.dma_start(t[:], d_hsT[k * 128:(k + 1) * 128, :])
                t_hsT.append(t)
                w = vsb.tile([128, cfg.D], F32R, tag=f"vw{k}", name=f"vw{k}")
                nc.sync.dma_start(w[:], d_vwT[k * 128:(k + 1) * 128, :])
                vwl.append(w)
            t_vb = sb1.tile([1, cfg.D], F32R, tag="vb")
            nc.sync.dma_start(t_vb[:], d_vwT[cfg.D:cfg.D + 1, :])
            t_onescol = sb1.tile([128, H], BF16, tag="onescol")
            nc.sync.dma_start(t_onescol[:], d_onescol[:, :])
            kw = []
            for k in range(KT):
                w = wsb.tile([128, cfg.D], F32R, tag=f"w{k}", name=f"kw{k}")
                nc.sync.dma_start(w[:], d_kwT[k * 128:(k + 1) * 128, :])
                kw.append(w)
            t_cos = sb1.tile([128, RL], F32, tag="cos")
            nc.sync.dma_start(t_cos[:], d_cosT2[:, :])
            t_sin = sb1.tile([128, RL], F32, tag="sin")
            nc.sync.dma_start(t_sin[:], d_sinT2[:, :])
            t_rotT = sb1.tile([128, 128], F32R, tag="rotT")
            nc.sync.dma_start(t_rotT[:], d_rotT[:, :])
            t_qb = sb1.tile([128, KT], F32, tag="qb")
            nc.sync.dma_start(t_qb[:], d_qb[:, :])
            qwl = []
            for k in range(KT):
                w = sb1.tile([128, cfg.D], F32R, tag=f"qw{k}", name=f"qw{k}")
                nc.sync.dma_start(w[:], d_qwT[k * 128:(k + 1) * 128, :])
                qwl.append(w)
            for t in range(NT):
                nc.sync.dma_start(t_EG[t][:], d_EG[t * 128:(t + 1) * 128, :])
            nc.sync.dma_start(t_ob[:], d_ob[:, :])
            for k in range(KT):
                nc.sync.dma_start(t_ow[k][:], d_owT[k * 128:(k + 1) * 128, :])

            # ---- projection / finalize emitters (scratch psum) ----
            def fin_half(dst, c0):
                """RoPE on dst[:, RS+c0 : RS+c0+512] in place."""
                p_rot = ps_s.tile([128, 512], F32, tag="qp", name="rot")
                nc.tensor.matmul(p_rot[:], t_rotT[:], dst[:, RS + c0:RS + c0 + 512],
                                 start=True, stop=True)
                tmp1 = sb1.tile([128, 512], F32, tag="tmp1", bufs=2)
                nc.vector.tensor_mul(tmp1[:], p_rot[:], t_sin[:, c0:c0 + 512])
                nc.gpsimd.tensor_mul(dst[:, RS + c0:RS + c0 + 512],
                                     dst[:, RS + c0:RS + c0 + 512],
                                     t_cos[:, c0:c0 + 512])
                nc.vector.tensor_add(dst[:, RS + c0:RS + c0 + 512],
                                     dst[:, RS + c0:RS + c0 + 512], tmp1[:])

            def fin_det(dst):
                nc.gpsimd.tensor_mul(dst[:, DS:DE], dst[:, DS:DE], t_tem[:])

            def proj_chunk(dst_tile, wlist, m, nb0, is_q):
                p = ps_s.tile([128, 512], F32, tag="qp")
                for k in range(KT):
                    nc.tensor.matmul(p[:, 0:SQB], wlist[k][:, m * 128:(m + 1) * 128],
                                     t_hsT[k][:, nb0:nb0 + SQB],
                                     start=(k == 0), stop=(k == KT - 1))
                if is_q:
                    nc.vector.tensor_scalar_add(dst_tile[:, nb0:nb0 + SQB],
                                                p[:, 0:SQB], t_qb[:, m:m + 1])
                else:
                    nc.scalar.copy(dst_tile[:, nb0:nb0 + SQB], p[:, 0:SQB])

            def v_chunk(mt, n0):
                p = ps_s.tile([128, 512], F32, tag="qp", name=f"vp{mt}_{n0}")
                nc.tensor.matmul(p[:, 0:SQB], t_ones[0:1, :], t_vb[0:1, n0:n0 + SQB],
                                 start=True, stop=False)
                for k in range(KT):
                    nc.tensor.matmul(p[:, 0:SQB], t_hsT[k][:, mt * 128:(mt + 1) * 128],
                                     vwl[k][:, n0:n0 + SQB],
                                     start=False, stop=(k == KT - 1))
                nh = SQB // HD  # heads covered by this chunk (6)
                h0 = n0 // HD
                vin = p[:, 0:SQB].rearrange("p (h j) -> p h j", h=nh)
                v3 = t_v[mt][:, h0 * 65:(h0 + nh) * 65].rearrange(
                    "p (h j) -> p h j", j=65)
                nc.scalar.activation(v3[:, :, 0:HD], vin, AF.Identity,
                                     bias=0.0, scale=1.0)

            def tile_units(m):
                """(tile_idx, closure) units building kT[m], qT[m] + finalize."""
                units = []
                for nb0 in range(0, SP, SQB):
                    units.append((m, lambda m=m, nb0=nb0: proj_chunk(t_kTf[m], kw, m, nb0, False)))
                units.append((m, lambda m=m: fin_half(t_kTf[m], 0)))
                units.append((m, lambda m=m: (fin_half(t_kTf[m], 512), fin_det(t_kTf[m]))))
                for nb0 in range(0, SP, SQB):
                    units.append((m, lambda m=m, nb0=nb0: proj_chunk(t_qTf[m], qwl, m, nb0, True)))
                units.append((m, lambda m=m: fin_half(t_qTf[m], 0)))
                units.append((m, lambda m=m: (fin_half(t_qTf[m], 512), fin_det(t_qTf[m]))))
                return units

            # ---- prelude: all of v, then kT/qT tile 0 ----
            for mt in range(NT):
                v_chunk(mt, 0)
                v_chunk(mt, SQB)
            oc3 = t_onescol[:, :].rearrange("p (h o) -> p h o", o=1)
            for mt in range(NT):
                v3 = t_v[mt][:, :].rearrange("p (h j) -> p h j", j=65)
                nc.vector.tensor_copy(v3[:, :, HD:65], oc3)
            for i in range(NE):
                nc.gpsimd.memset(t_e[i][:, S:SP], 0.0)
            vst.close()  # v weights no longer needed
            for _, u in tile_units(0):
                u()

            # remaining projection work, popped between attention steps
            fillers = []
            for m in range(1, KT):
                fillers.extend(tile_units(m))
            fillers.reverse()  # pop from the end = logical order

            def pop_filler():
                if fillers:
                    fillers.pop()[1]()

            def drain_to_tile(mi):
                while fillers and any(i <= mi for i, _ in fillers):
                    fillers.pop()[1]()

            # ---- attention ----
            xt = ps_X.tile([128, 512], F32, tag="X")
            ei = 0
            step = 0
            pend_ctx = None
            pB_open = True

            def emit_ctx(h, skt, e, c73):
                for mt in range(7):
                    nc.tensor.matmul(c73[:, mt, 0:65],
                                     e[:, mt * 128:(mt + 1) * 128],
                                     t_v[skt][:, h * 65:h * 65 + 65],
                                     start=(skt == 0), stop=(skt == NT - 1))
                for mt in (7, 8):
                    o = XC + CSW * (mt - 7)
                    nc.tensor.matmul(xt[:, o:o + 65],
                                     e[:, mt * 128:(mt + 1) * 128],
                                     t_v[skt][:, h * 65:h * 65 + 65],
                                     start=(skt == 0), stop=(skt == NT - 1))

            for h in range(H):
                hp, hr = h // 2, (h % 2) * 64
                drain_to_tile(hp)
                if not fillers and pB_open:
                    pB.close()
                    pB_open = False
                p_c7 = ps_c7.tile([128, 512], F32, tag="c7")
                c73 = p_c7[:, 0:CSW * 7].rearrange("p (m w) -> p m w", w=CSW)
                for skt in range(NT):
                    sc = (ps_mA if step % 2 == 0 else ps_mB).tile(
                        [128, 1024], F32, tag="m")
                    ti = step % 3
                    for c0, cw in ((0, 512), (512, 512)):
                        nc.tensor.matmul(sc[:, c0:c0 + cw],
                                         t_kTf[hp][hr:hr + 64, skt * 128:(skt + 1) * 128],
                                         t_qTf[hp][hr:hr + 64, c0:c0 + cw],
                                         start=True, stop=True)
                    nc.tensor.matmul(xt[:, ti * TW:(ti + 1) * TW],
                                     t_kTf[hp][hr:hr + 64, skt * 128:(skt + 1) * 128],
                                     t_qTf[hp][hr:hr + 64, 1024:S],
                                     start=True, stop=True)
                    e = t_e[ei % NE]
                    ei += 1
                    nc.scalar.activation(e[:, 0:1024], sc[:, 0:1024], AF.Exp,
                                         bias=0.0, scale=1.0)
                    nc.scalar.activation(e[:, 1024:S], xt[:, ti * TW:(ti + 1) * TW],
                                         AF.Exp, bias=0.0, scale=1.0)
                    nc.vector.tensor_mul(e[:, 0:S], e[:, 0:S], t_EG[skt][:, 0:S])
                    if pB_open and step % cfg.FILLER_EVERY == 0:
                        pop_filler()
                    if pend_ctx is not None:
                        emit_ctx(*pend_ctx)
                    pend_ctx = (h, skt, e, c73)
                    step += 1
                emit_ctx(*pend_ctx)
                pend_ctx = None
                rec = t_rec[h % 2]
                r3 = rec[:, 0:7].rearrange("p (m o) -> p m o", o=1)
                nc.vector.reciprocal(r3[:, :, :], c73[:, :, 64:65])
                nc.vector.reciprocal(rec[:, 7:8], xt[:, XC + 64:XC + 65])
                nc.vector.reciprocal(rec[:, 8:9], xt[:, XC + CSW + 64:XC + CSW + 65])
                for mt in range(7):
                    nc.gpsimd.tensor_scalar_mul(
                        t_ctx[mt][:, h * HD:(h + 1) * HD],
                        c73[:, mt, 0:HD], rec[:, mt:mt + 1])
                for mt in (7, 8):
                    o = XC + CSW * (mt - 7)
                    nc.gpsimd.tensor_scalar_mul(
                        t_ctx[mt][:, h * HD:(h + 1) * HD],
                        xt[:, o:o + HD], rec[:, mt:mt + 1])
                if h % 2 == 1:
                    for mt in range(NT):
                        nc.sync.dma_start_transpose(
                            t_ctxT[hp][:, mt * 128:(mt + 1) * 128],
                            t_ctx[mt][:, hp * 128:(hp + 1) * 128])
            if pB_open:
                pB.close()
            apsum.close()  # free attention psum before the tail pool opens

            # ================== tail: output projection =====================
            with ExitStack() as p4:
                sb4 = p4.enter_context(tc.tile_pool(name="sb4", bufs=1))
                ps_o = p4.enter_context(tc.tile_pool(name="ps_o", bufs=3, space="PSUM"))
                for mt in range(NT):
                    t_out = sb4.tile([128, cfg.D], F32, tag="out", bufs=3)
                    p_o = ps_o.tile([128, cfg.D], F32, tag="po")
                    for n0 in range(0, cfg.D, 512):
                        nw = min(512, cfg.D - n0)
                        for k in range(KT):
                            nc.tensor.matmul(p_o[:, n0:n0 + nw],
                                             t_ctxT[k][:, mt * 128:(mt + 1) * 128],
                                             t_ow[k][:, n0:n0 + nw],
                                             start=(k == 0), stop=(k == KT - 1))
                    nc.vector.tensor_add(t_out[:], p_o[:], t_ob[:])
                    nc.sync.dma_start(d_out[mt * 128:(mt + 1) * 128, :], t_out[:])

    nc.compile()
    return nc


# ---------------- host-side prep + dispatch --------------------------------


def _host_prep(cfg: CFG, hidden_states, q_w, q_b, k_w, v_w, v_b, o_w, o_b,
               cos, sin, ph, pw, gate):
    KT, SP, H, HD = cfg.KT, cfg.SP, cfg.H, cfg.HD
    D, S = cfg.D, cfg.S
    DET = cfg.DET_END - cfg.DET_START
    half = HD // 2
    scale = HD ** -0.5

    shared = {}
    shared["qwT"] = round_f32r(q_w.T * scale)
    shared["kwT"] = round_f32r(k_w.T)
    vwT = np.zeros(((KT + 1) * 128, D), np.float32)
    vwT[:D] = v_w.T
    vwT[D] = v_b
    shared["vwT"] = round_f32r(vwT)
    shared["owT"] = round_f32r(o_w.T)
    # multiplicative gate: exp(gate^T), pad keys killed via exp(GATE_NEG)
    gateT = np.full((SP, SP), cfg.GATE_NEG, np.float32)
    gateT[:S, :S] = gate[0, 0].T
    shared["EG"] = to_bf16(np.exp(gateT))
    # q bias pre-scaled, laid out [128, KT]
    qb = (q_b.astype(np.float32) * scale).reshape(KT, 128).T
    shared["qb"] = np.ascontiguousarray(qb)
    shared["ob"] = np.broadcast_to(o_b.astype(np.float32)[None, :], (128, D)).copy()
    # rope tables: [128, RL] = two stacked head-copies of cos/sin transposed
    cosT = cos.T.astype(np.float32)                       # [HD, RL]
    sinT = sin.T.astype(np.float32)
    shared["cosT2"] = np.vstack([cosT, cosT]).astype(np.float32)
    shared["sinT2"] = np.vstack([sinT, sinT]).astype(np.float32)
    # rotation matrix R (rotate_half along the hd partition dim), applied as
    # R @ x via lhsT = R.T; R spans two stacked heads per 128-partition tile
    R = np.zeros((128, 128), np.float32)
    for blk in range(2):
        o = blk * HD
        for j in range(half):
            R[o + j, o + half + j] = -1.0
            R[o + half + j, o + j] = 1.0
    shared["rotT"] = round_f32r(R.T)
    shared["ones"] = round_f32r(np.ones((1, 128), np.float32))
    shared["onescol"] = to_bf16(np.ones((128, H), np.float32))
    maska = np.zeros((1, 128), np.float32)
    maskb = np.zeros((1, 128), np.float32)
    for p in range(128):
        if (p % HD) < half:
            maska[0, p] = 1.0
        else:
            maskb[0, p] = 1.0
    shared["masks"] = round_f32r(np.concatenate([maska, maskb], axis=1))
    shared["ph"] = ph.astype(np.float32).reshape(1, DET)
    shared["pw"] = pw.astype(np.float32).reshape(1, DET)

    in_maps = []
    for c in range(cfg.N_CORES):
        hsT = np.zeros((D, SP), np.float32)
        hsT[:, :S] = hidden_states[c].T
        m = dict(shared)
        m["hsT"] = to_bf16(hsT)
        in_maps.append(m)
    return in_maps


_NC_CACHE = {}


def kernel(hidden_states, q_w, q_b, k_w, v_w, v_b, o_w, o_b,
           cos, sin, ph, pw, gate,
           rope_start=5, rope_end=1029, det_start=1029, det_end=1129):
    cfg = CFG()
    in_maps = _host_prep(cfg, np.asarray(hidden_states, np.float32),
                         np.asarray(q_w, np.float32), np.asarray(q_b, np.float32),
                         np.asarray(k_w, np.float32), np.asarray(v_w, np.float32),
                         np.asarray(v_b, np.float32), np.asarray(o_w, np.float32),
                         np.asarray(o_b, np.float32), np.asarray(cos, np.float32),
                         np.asarray(sin, np.float32), np.asarray(ph, np.float32),
                         np.asarray(pw, np.float32), np.asarray(gate, np.float32))
    if "nc" not in _NC_CACHE:
        _NC_CACHE["nc"] = build_nc(cfg)
    nc = _NC_CACHE["nc"]
    res = run_bass_kernel_spmd(nc, in_maps, list(range(cfg.N_CORES)))
    out = np.stack([res.results[c]["out"][:cfg.S] for c in range(cfg.N_CORES)])
    return out.astype(np.float32)


# revision 19
# speedup vs baseline: 1.3540x; 1.0084x over previous
"""DINOv3 ViT attention (RoPE + det-temp scaling + additive gate) on 8 TRN2 cores.

Sharding: pure data-parallel over batch (B=8 -> 1 batch element per core).
Weights / gate / rope tables replicated. No collectives.

v3 design (engine budget per the TimelineSim cost model):
  - gate folded multiplicatively: EG = exp(gate^T) precomputed on host (bf16);
    after the ACT exp of raw scores, one DVE bf16 multiply applies it
    (replaces the per-head identity-matmul gate copies on PE).
  - scores per (head, skt): qT bf16 (moving operand -> 1 cyc/col at any
    width), kT bf16 (stationary), full-sq psum [128, 1152] in 3 chunks.
  - exp on ACT in 1-2 wide ops per (head, skt) (amortizes the ~185ns per-op
    access-latency penalty).
  - ctx flipped: out[sq_tile, 65] = e'^T @ v  (lhsT = e' bf16), N=65 per
    matmul -> half the PE columns of the [hd, sq] orientation; the v ones
    column lands the softmax denominator as a per-partition column, so
    normalization is one strided DVE reciprocal + 9 GpSimd scalar muls.
  - ctx [sq, hd] bf16 -> ctxT [hd, sq] via DMA XBAR transposes (idle engine).
  - phase merge: attention for head pair p starts as soon as kT/qT tile p is
    finalized; remaining projection/RoPE work is emitted as PE filler between
    attention steps.  While projection psum pools are open the score psum is
    single-buffered (8-bank budget) and the exp is split in two so the
    next-step score matmuls unblock early; once projections finish, a second
    score psum opens and exps run whole.
"""
import numpy as np
from contextlib import ExitStack

import ml_dtypes
import concourse.bacc as bacc
import concourse.mybir as mybir
import concourse.tile as tile
from concourse.bass_utils import run_bass_kernel_spmd

F32 = mybir.dt.float32
F32R = mybir.dt.float32r
BF16 = mybir.dt.bfloat16
AF = mybir.ActivationFunctionType

# ---------------- problem config (hardcoded per harness contract) ------------


class CFG:
    B = 8
    S = 1129
    SP = 1152            # padded S (9 * 128)
    D = 768
    H = 12
    HD = 64
    ROPE_START = 5
    ROPE_END = 1029
    DET_START = 1029
    DET_END = 1129
    P_SCALE = 2.0
    N_CORES = 8
    SQB = 384            # projection psum chunk
    GATE_NEG = -30.0     # gate value for pad keys: exp(-30) ~ 9e-14
    CTX_STRIDE = 74      # ctx psum window stride (65-wide windows, no
                         # 512-col psum bank crossings for 9 windows)
    FILLER_EVERY = 2     # pop one projection filler every N attention steps

    @property
    def KT(self):
        return self.D // 128          # dout/din 128-tiles (6)

    @property
    def NT(self):
        return self.SP // 128         # s 128-tiles (9)

    @property
    def ROPE_LEN(self):
        return self.ROPE_END - self.ROPE_START


def round_f32r(x: np.ndarray) -> np.ndarray:
    """Round fp32 to the fp32r format (11 mantissa bits, RNE)."""
    b = np.ascontiguousarray(x, dtype=np.float32).view(np.uint32)
    low = b & np.uint32(0xFFF)
    b = b & np.uint32(0xFFFFF000)
    rnd = (low > 0x800) | ((low == 0x800) & (((b >> 12) & 1) != 0))
    b = b + (rnd.astype(np.uint32) << 12)
    return b.view(np.float32)


def to_bf16(x: np.ndarray) -> np.ndarray:
    return np.ascontiguousarray(x, dtype=np.float32).astype(ml_dtypes.bfloat16)


# ---------------- device program ------------------------------------------


def build_nc(cfg: CFG):
    nc = bacc.Bacc(trn_type="TRN2", target_bir_lowering=False, debug=False)
    KT, NT, SQB, SP = cfg.KT, cfg.NT, cfg.SQB, cfg.SP
    H, HD = cfg.H, cfg.HD
    RS, RE, DS, DE = cfg.ROPE_START, cfg.ROPE_END, cfg.DET_START, cfg.DET_END
    RL = cfg.ROPE_LEN
    DET = DE - DS
    S = cfg.S
    CS = cfg.CTX_STRIDE

    # ---- dram parameters (per core) ----
    d_hsT = nc.dram_tensor("hsT", [cfg.D, SP], BF16, kind="ExternalInput").ap()
    d_qwT = nc.dram_tensor("qwT", [cfg.D, cfg.D], F32R, kind="ExternalInput").ap()
    d_kwT = nc.dram_tensor("kwT", [cfg.D, cfg.D], F32R, kind="ExternalInput").ap()
    d_vwT = nc.dram_tensor("vwT", [(KT + 1) * 128, cfg.D], F32R, kind="ExternalInput").ap()
    d_owT = nc.dram_tensor("owT", [cfg.D, cfg.D], F32R, kind="ExternalInput").ap()
    d_EG = nc.dram_tensor("EG", [SP, SP], BF16, kind="ExternalInput").ap()
    d_qb = nc.dram_tensor("qb", [128, KT], F32, kind="ExternalInput").ap()
    d_ob = nc.dram_tensor("ob", [128, cfg.D], F32, kind="ExternalInput").ap()
    d_cosT2 = nc.dram_tensor("cosT2", [128, RL], F32, kind="ExternalInput").ap()
    d_sinT2 = nc.dram_tensor("sinT2", [128, RL], F32, kind="ExternalInput").ap()
    d_rotT = nc.dram_tensor("rotT", [128, 128], F32R, kind="ExternalInput").ap()
    d_ones = nc.dram_tensor("ones", [1, 128], F32R, kind="ExternalInput").ap()
    d_onescol = nc.dram_tensor("onescol", [128, H], BF16, kind="ExternalInput").ap()
    d_masks = nc.dram_tensor("masks", [1, 256], F32R, kind="ExternalInput").ap()
    d_ph = nc.dram_tensor("ph", [1, DET], F32, kind="ExternalInput").ap()
    d_pw = nc.dram_tensor("pw", [1, DET], F32, kind="ExternalInput").ap()
    d_out = nc.dram_tensor("out", [SP, cfg.D], F32, kind="ExternalOutput").ap()

    with tile.TileContext(nc) as tc, ExitStack() as gctx:
        # ---------------- global sbuf (spans the whole kernel) --------------
        gsb = gctx.enter_context(tc.tile_pool(name="gsb", bufs=1))

        t_ones = gsb.tile([1, 128], F32R, tag="ones")
        nc.sync.dma_start(t_ones[:], d_ones[:, :])

        # persistent activations
        t_qTf = [gsb.tile([128, SP], BF16, tag=f"qTf{m}", name=f"qTf{m}") for m in range(KT)]
        t_kTf = [gsb.tile([128, SP], BF16, tag=f"kTf{m}", name=f"kTf{m}") for m in range(KT)]
        t_v = [gsb.tile([128, H * 65], BF16, tag=f"v{t}", name=f"v{t}") for t in range(NT)]
        t_EG = [gsb.tile([128, SP], BF16, tag=f"eg{t}", name=f"eg{t}") for t in range(NT)]
        NE = 4
        t_e = [gsb.tile([128, SP], BF16, tag=f"e{i}", name=f"e{i}") for i in range(NE)]
        t_ctx = [gsb.tile([128, cfg.D], BF16, tag=f"ctx{t}", name=f"ctx{t}") for t in range(NT)]
        t_ctxT = [gsb.tile([128, SP], BF16, tag=f"ctxT{k}", name=f"ctxT{k}") for k in range(KT)]
        t_rec = [gsb.tile([128, NT], F32, tag=f"rec{i}", name=f"rec{i}") for i in range(2)]
        t_tem = gsb.tile([128, DET], F32, tag="tem")

        # ---- det temperature pattern (standalone psum stack) ----
        with ExitStack() as ptem:
            sbt = ptem.enter_context(tc.tile_pool(name="sbt", bufs=1))
            ps_tem = ptem.enter_context(tc.tile_pool(name="ps_tem", bufs=1, space="PSUM"))
            t_ms = sbt.tile([1, 256], F32R, tag="ms")
            nc.sync.dma_start(t_ms[:], d_masks[:, :])
            t_ph = sbt.tile([1, DET], F32, tag="ph")
            nc.sync.dma_start(t_ph[:], d_ph[:, :])
            t_pw = sbt.tile([1, DET], F32, tag="pw")
            nc.sync.dma_start(t_pw[:], d_pw[:, :])
            t_eh = sbt.tile([1, DET], F32R, tag="eh")
            nc.scalar.activation(t_eh[:], t_ph[:], AF.Exp, bias=0.0, scale=cfg.P_SCALE)
            t_ew = sbt.tile([1, DET], F32R, tag="ew")
            nc.scalar.activation(t_ew[:], t_pw[:], AF.Exp, bias=0.0, scale=cfg.P_SCALE)
            p_tem = ps_tem.tile([128, DET], F32, tag="tem")
            nc.tensor.matmul(p_tem[:], t_ms[0:1, 0:128], t_eh[:], start=True, stop=False)
            nc.tensor.matmul(p_tem[:], t_ms[0:1, 128:256], t_ew[:], start=False, stop=True)
            nc.vector.tensor_copy(t_tem[:], p_tem[:])

        # ==================== merged projections + attention ================
        # psum layout (8 banks, scores double-buffered while projection
        # pools are open):
        #   mA, mB: [128,1024] score mains (2 banks each)
        #   X:      [128,512] shared bank: 3 rotating 105-col score sq-tails
        #           + ctx windows for sq tiles 7,8 (at 320, 393)
        #   c7:     [128,512] ctx windows for sq tiles 0..6 (stride 73)
        #   scratch:[128,512] x2 projection chunks / rope psum
        TW = S - 1024                    # 105: sq tail width
        XC = 320                         # ctx78 base offset inside X
        CSW = 73                         # ctx window stride
        with ExitStack() as att:
            sb3 = att.enter_context(tc.tile_pool(name="sb3", bufs=1))
            t_ob = sb3.tile([128, cfg.D], F32, tag="ob")
            t_ow = [sb3.tile([128, cfg.D], F32R, tag=f"ow{k}", name=f"ow{k}")
                    for k in range(KT)]
            apsum = att.enter_context(ExitStack())
            ps_mA = apsum.enter_context(tc.tile_pool(name="ps_mA", bufs=1, space="PSUM"))
            ps_mB = apsum.enter_context(tc.tile_pool(name="ps_mB", bufs=1, space="PSUM"))
            ps_X = apsum.enter_context(tc.tile_pool(name="ps_X", bufs=1, space="PSUM"))
            ps_c7 = apsum.enter_context(tc.tile_pool(name="ps_c7", bufs=1, space="PSUM"))

            pB = ExitStack()
            sb1 = pB.enter_context(tc.tile_pool(name="sb1", bufs=1))
            wsb = pB.enter_context(tc.tile_pool(name="wsb", bufs=1))
            ps_s = pB.enter_context(tc.tile_pool(name="ps_s", bufs=2, space="PSUM"))

            # ---- loads: hsT/vw first (v-projection leads), then kw, qw ----
            vst = ExitStack()
            vsb = vst.enter_context(tc.tile_pool(name="vsb", bufs=1))

            t_hsT = []
            vwl = []
            for k in range(KT):
                t = sb1.tile([128, SP], BF16, tag=f"hsT{k}", name=f"hsT{k}")
                nc.sync.dma_start(t[:], d_hsT[k * 128:(k + 1) * 128, :])
                t_hsT.append(t)
                w = vsb.tile([128, cfg.D], F32R, tag=f"vw{k}", name=f"vw{k}")
                nc.sync.dma_start(w[:], d_vwT[k * 128:(k + 1) * 128, :])
                vwl.append(w)
            t_vb = sb1.tile([1, cfg.D], F32R, tag="vb")
            nc.sync.dma_start(t_vb[:], d_vwT[cfg.D:cfg.D + 1, :])
            t_onescol = sb1.tile([128, H], BF16, tag="onescol")
            nc.sync.dma_start(t_onescol[:], d_onescol[:, :])
            kw = []
            for k in range(KT):
                w = wsb.tile([128, cfg.D], F32R, tag=f"w{k}", name=f"kw{k}")
                nc.sync.dma_start(w[:], d_kwT[k * 128:(k + 1) * 128, :])
                kw.append(w)
            t_cos = sb1.tile([128, RL], F32, tag="cos")
            nc.sync.dma_start(t_cos[:], d_cosT2[:, :])
            t_sin = sb1.tile([128, RL], F32, tag="sin")
            nc.sync.dma_start(t_sin[:], d_sinT2[:, :])
            t_rotT = sb1.tile([128, 128], F32R, tag="rotT")
            nc.sync.dma_start(t_rotT[:], d_rotT[:, :])
            t_qb = sb1.tile([128, KT], F32, tag="qb")
            nc.sync.dma_start(t_qb[:], d_qb[:, :])
            qwl = []
            for k in range(KT):
                w = sb1.tile([128, cfg.D], F32R, tag=f"qw{k}", name=f"qw{k}")
                nc.sync.dma_start(w[:], d_qwT[k * 128:(k + 1) * 128, :])
                qwl.append(w)
            for t in range(NT):
                nc.sync.dma_start(t_EG[t][:], d_EG[t * 128:(t + 1) * 128, :])
            nc.sync.dma_start(t_ob[:], d_ob[:, :])
            for k in range(KT):
                nc.sync.dma_start(t_ow[k][:], d_owT[k * 128:(k + 1) * 128, :])

            # ---- projection / finalize emitters (scratch psum) ----
            def fin_half(dst, c0):
                """RoPE on dst[:, RS+c0 : RS+c0+512] in place."""
                p_rot = ps_s.tile([128, 512], F32, tag="qp", name="rot")
                nc.tensor.matmul(p_rot[:], t_rotT[:], dst[:, RS + c0:RS + c0 + 512],
                                 start=True, stop=True)
                tmp1 = sb1.tile([128, 512], F32, tag="tmp1", bufs=2)
                nc.vector.tensor_mul(tmp1[:], p_rot[:], t_sin[:, c0:c0 + 512])
                nc.gpsimd.tensor_mul(dst[:, RS + c0:RS + c0 + 512],
                                     dst[:, RS + c0:RS + c0 + 512],
                                     t_cos[:, c0:c0 + 512])
                nc.vector.tensor_add(dst[:, RS + c0:RS + c0 + 512],
                                     dst[:, RS + c0:RS + c0 + 512], tmp1[:])

            def fin_det(dst):
                nc.gpsimd.tensor_mul(dst[:, DS:DE], dst[:, DS:DE], t_tem[:])

            def proj_chunk(dst_tile, wlist, m, nb0, is_q):
                p = ps_s.tile([128, 512], F32, tag="qp")
                for k in range(KT):
                    nc.tensor.matmul(p[:, 0:SQB], wlist[k][:, m * 128:(m + 1) * 128],
                                     t_hsT[k][:, nb0:nb0 + SQB],
                                     start=(k == 0), stop=(k == KT - 1))
                if is_q:
                    nc.vector.tensor_scalar_add(dst_tile[:, nb0:nb0 + SQB],
                                                p[:, 0:SQB], t_qb[:, m:m + 1])
                else:
                    nc.scalar.copy(dst_tile[:, nb0:nb0 + SQB], p[:, 0:SQB])

            def v_chunk(mt, n0):
                p = ps_s.tile([128, 512], F32, tag="qp", name=f"vp{mt}_{n0}")
                nc.tensor.matmul(p[:, 0:SQB], t_ones[0:1, :], t_vb[0:1, n0:n0 + SQB],
                                 start=True, stop=False)
                for k in range(KT):
                    nc.tensor.matmul(p[:, 0:SQB], t_hsT[k][:, mt * 128:(mt + 1) * 128],
                                     vwl[k][:, n0:n0 + SQB],
                                     start=False, stop=(k == KT - 1))
                nh = SQB // HD  # heads covered by this chunk (6)
                h0 = n0 // HD
                vin = p[:, 0:SQB].rearrange("p (h j) -> p h j", h=nh)
                v3 = t_v[mt][:, h0 * 65:(h0 + nh) * 65].rearrange(
                    "p (h j) -> p h j", j=65)
                nc.scalar.activation(v3[:, :, 0:HD], vin, AF.Identity,
                                     bias=0.0, scale=1.0)

            def tile_units(m):
                """(tile_idx, closure) units building kT[m], qT[m] + finalize."""
                units = []
                for nb0 in range(0, SP, SQB):
                    units.append((m, lambda m=m, nb0=nb0: proj_chunk(t_kTf[m], kw, m, nb0, False)))
                units.append((m, lambda m=m: fin_half(t_kTf[m], 0)))
                units.append((m, lambda m=m: (fin_half(t_kTf[m], 512), fin_det(t_kTf[m]))))
                for nb0 in range(0, SP, SQB):
                    units.append((m, lambda m=m, nb0=nb0: proj_chunk(t_qTf[m], qwl, m, nb0, True)))
                units.append((m, lambda m=m: fin_half(t_qTf[m], 0)))
                units.append((m, lambda m=m: (fin_half(t_qTf[m], 512), fin_det(t_qTf[m]))))
                return units

            # ---- prelude: all of v, then kT/qT tile 0 ----
            for mt in range(NT):
                v_chunk(mt, 0)
                v_chunk(mt, SQB)
            oc3 = t_onescol[:, :].rearrange("p (h o) -> p h o", o=1)
            for mt in range(NT):
                v3 = t_v[mt][:, :].rearrange("p (h j) -> p h j", j=65)
                nc.vector.tensor_copy(v3[:, :, HD:65], oc3)
            for i in range(NE):
                nc.gpsimd.memset(t_e[i][:, S:SP], 0.0)
            vst.close()  # v weights no longer needed
            for _, u in tile_units(0):
                u()

            # remaining projection work, popped between attention steps
            fillers = []
            for m in range(1, KT):
                fillers.extend(tile_units(m))
            fillers.reverse()  # pop from the end = logical order

            def pop_filler():
                if fillers:
                    fillers.pop()[1]()

            def drain_to_tile(mi):
                while fillers and any(i <= mi for i, _ in fillers):
                    fillers.pop()[1]()

            # ---- attention ----
            xt = ps_X.tile([128, 512], F32, tag="X")
            ei = 0
            step = 0
            pend_ctx = None
            pB_open = True

            def emit_ctx(h, skt, e, c73):
                for mt in range(7):
                    nc.tensor.matmul(c73[:, mt, 0:65],
                                     e[:, mt * 128:(mt + 1) * 128],
                                     t_v[skt][:, h * 65:h * 65 + 65],
                                     start=(skt == 0), stop=(skt == NT - 1))
                for mt in (7, 8):
                    o = XC + CSW * (mt - 7)
                    nc.tensor.matmul(xt[:, o:o + 65],
                                     e[:, mt * 128:(mt + 1) * 128],
                                     t_v[skt][:, h * 65:h * 65 + 65],
                                     start=(skt == 0), stop=(skt == NT - 1))

            def head_end(h, c73, e_last):
                """Flush trailing ctx, normalize, transpose for head h."""
                emit_ctx(h, NT - 1, e_last, c73)
                rec = t_rec[h % 2]
                r3 = rec[:, 0:7].rearrange("p (m o) -> p m o", o=1)
                nc.vector.reciprocal(r3[:, :, :], c73[:, :, 64:65])
                nc.vector.reciprocal(rec[:, 7:8], xt[:, XC + 64:XC + 65])
                nc.vector.reciprocal(rec[:, 8:9], xt[:, XC + CSW + 64:XC + CSW + 65])
                for mt in range(7):
                    nc.gpsimd.tensor_scalar_mul(
                        t_ctx[mt][:, h * HD:(h + 1) * HD],
                        c73[:, mt, 0:HD], rec[:, mt:mt + 1])
                for mt in (7, 8):
                    o = XC + CSW * (mt - 7)
                    nc.gpsimd.tensor_scalar_mul(
                        t_ctx[mt][:, h * HD:(h + 1) * HD],
                        xt[:, o:o + HD], rec[:, mt:mt + 1])
                if h % 2 == 1:
                    for mt in range(NT):
                        nc.sync.dma_start_transpose(
                            t_ctxT[h // 2][:, mt * 128:(mt + 1) * 128],
                            t_ctx[mt][:, (h // 2) * 128:(h // 2 + 1) * 128])

            prev_head_end = None  # deferred so next head's scores aren't blocked
            for h in range(H):
                hp, hr = h // 2, (h % 2) * 64
                drain_to_tile(hp)
                if not fillers and pB_open:
                    pB.close()
                    pB_open = False
                p_c7 = ps_c7.tile([128, 512], F32, tag="c7")
                c73 = p_c7[:, 0:CSW * 7].rearrange("p (m w) -> p m w", w=CSW)
                for skt in range(NT):
                    sc = (ps_mA if step % 2 == 0 else ps_mB).tile(
                        [128, 1024], F32, tag="m")
                    ti = step % 3
                    for c0, cw in ((0, 512), (512, 512)):
                        nc.tensor.matmul(sc[:, c0:c0 + cw],
                                         t_kTf[hp][hr:hr + 64, skt * 128:(skt + 1) * 128],
                                         t_qTf[hp][hr:hr + 64, c0:c0 + cw],
                                         start=True, stop=True)
                    nc.tensor.matmul(xt[:, ti * TW:(ti + 1) * TW],
                                     t_kTf[hp][hr:hr + 64, skt * 128:(skt + 1) * 128],
                                     t_qTf[hp][hr:hr + 64, 1024:S],
                                     start=True, stop=True)
                    e = t_e[ei % NE]
                    ei += 1
                    nc.scalar.activation(e[:, 0:1024], sc[:, 0:1024], AF.Exp,
                                         bias=0.0, scale=1.0)
                    nc.scalar.activation(e[:, 1024:S], xt[:, ti * TW:(ti + 1) * TW],
                                         AF.Exp, bias=0.0, scale=1.0)
                    nc.vector.tensor_mul(e[:, 0:S], e[:, 0:S], t_EG[skt][:, 0:S])
                    if prev_head_end is not None:
                        prev_head_end()
                        prev_head_end = None
                    elif pB_open and step % cfg.FILLER_EVERY == 0:
                        pop_filler()
                    if pend_ctx is not None:
                        emit_ctx(*pend_ctx)
                    if skt < NT - 1:
                        pend_ctx = (h, skt, e, c73)
                    else:
                        # last step's ctx is flushed inside head_end
                        pend_ctx = None
                        prev_head_end = (lambda h=h, c73=c73, e=e:
                                         head_end(h, c73, e))
                    step += 1
            prev_head_end()
            if pB_open:
                pB.close()
            apsum.close()  # free attention psum before the tail pool opens

            # ================== tail: output projection =====================
            with ExitStack() as p4:
                sb4 = p4.enter_context(tc.tile_pool(name="sb4", bufs=1))
                ps_o = p4.enter_context(tc.tile_pool(name="ps_o", bufs=3, space="PSUM"))
                for mt in range(NT):
                    t_out = sb4.tile([128, cfg.D], F32, tag="out", bufs=3)
                    p_o = ps_o.tile([128, cfg.D], F32, tag="po")
                    for n0 in range(0, cfg.D, 512):
                        nw = min(512, cfg.D - n0)
                        for k in range(KT):
                            nc.tensor.matmul(p_o[:, n0:n0 + nw],
                                             t_ctxT[k][:, mt * 128:(mt + 1) * 128],
                                             t_ow[k][:, n0:n0 + nw],
                                             start=(k == 0), stop=(k == KT - 1))
                    nc.vector.tensor_add(t_out[:], p_o[:], t_ob[:])
                    nc.sync.dma_start(d_out[mt * 128:(mt + 1) * 128, :], t_out[:])

    nc.compile()
    return nc


# ---------------- host-side prep + dispatch --------------------------------


def _host_prep(cfg: CFG, hidden_states, q_w, q_b, k_w, v_w, v_b, o_w, o_b,
               cos, sin, ph, pw, gate):
    KT, SP, H, HD = cfg.KT, cfg.SP, cfg.H, cfg.HD
    D, S = cfg.D, cfg.S
    DET = cfg.DET_END - cfg.DET_START
    half = HD // 2
    scale = HD ** -0.5

    shared = {}
    shared["qwT"] = round_f32r(q_w.T * scale)
    shared["kwT"] = round_f32r(k_w.T)
    vwT = np.zeros(((KT + 1) * 128, D), np.float32)
    vwT[:D] = v_w.T
    vwT[D] = v_b
    shared["vwT"] = round_f32r(vwT)
    shared["owT"] = round_f32r(o_w.T)
    # multiplicative gate: exp(gate^T), pad keys killed via exp(GATE_NEG)
    gateT = np.full((SP, SP), cfg.GATE_NEG, np.float32)
    gateT[:S, :S] = gate[0, 0].T
    shared["EG"] = to_bf16(np.exp(gateT))
    # q bias pre-scaled, laid out [128, KT]
    qb = (q_b.astype(np.float32) * scale).reshape(KT, 128).T
    shared["qb"] = np.ascontiguousarray(qb)
    shared["ob"] = np.broadcast_to(o_b.astype(np.float32)[None, :], (128, D)).copy()
    # rope tables: [128, RL] = two stacked head-copies of cos/sin transposed
    cosT = cos.T.astype(np.float32)                       # [HD, RL]
    sinT = sin.T.astype(np.float32)
    shared["cosT2"] = np.vstack([cosT, cosT]).astype(np.float32)
    shared["sinT2"] = np.vstack([sinT, sinT]).astype(np.float32)
    # rotation matrix R (rotate_half along the hd partition dim), applied as
    # R @ x via lhsT = R.T; R spans two stacked heads per 128-partition tile
    R = np.zeros((128, 128), np.float32)
    for blk in range(2):
        o = blk * HD
        for j in range(half):
            R[o + j, o + half + j] = -1.0
            R[o + half + j, o + j] = 1.0
    shared["rotT"] = round_f32r(R.T)
    shared["ones"] = round_f32r(np.ones((1, 128), np.float32))
    shared["onescol"] = to_bf16(np.ones((128, H), np.float32))
    maska = np.zeros((1, 128), np.float32)
    maskb = np.zeros((1, 128), np.float32)
    for p in range(128):
        if (p % HD) < half:
            maska[0, p] = 1.0
        else:
            maskb[0, p] = 1.0
    shared["masks"] = round_f32r(np.concatenate([maska, maskb], axis=1))
    shared["ph"] = ph.astype(np.float32).reshape(1, DET)
    shared["pw"] = pw.astype(np.float32).reshape(1, DET)

    in_maps = []
    for c in range(cfg.N_CORES):
        hsT = np.zeros((D, SP), np.float32)
        hsT[:, :S] = hidden_states[c].T
        m = dict(shared)
        m["hsT"] = to_bf16(hsT)
        in_maps.append(m)
    return in_maps


_NC_CACHE = {}


def kernel(hidden_states, q_w, q_b, k_w, v_w, v_b, o_w, o_b,
           cos, sin, ph, pw, gate,
           rope_start=5, rope_end=1029, det_start=1029, det_end=1129):
    cfg = CFG()
    in_maps = _host_prep(cfg, np.asarray(hidden_states, np.float32),
                         np.asarray(q_w, np.float32), np.asarray(q_b, np.float32),
                         np.asarray(k_w, np.float32), np.asarray(v_w, np.float32),
                         np.asarray(v_b, np.float32), np.asarray(o_w, np.float32),
                         np.asarray(o_b, np.float32), np.asarray(cos, np.float32),
                         np.asarray(sin, np.float32), np.asarray(ph, np.float32),
                         np.asarray(pw, np.float32), np.asarray(gate, np.float32))
    if "nc" not in _NC_CACHE:
        _NC_CACHE["nc"] = build_nc(cfg)
    nc = _NC_CACHE["nc"]
    res = run_bass_kernel_spmd(nc, in_maps, list(range(cfg.N_CORES)))
    out = np.stack([res.results[c]["out"][:cfg.S] for c in range(cfg.N_CORES)])
    return out.astype(np.float32)
